# revision 25
# baseline (speedup 1.0000x reference)
"""KA-GNN (Fourier-KAN message passing) on 8 Trainium2 NeuronCores — v3.

Sharding: nodes/edges partitioned by destination across 8 cores, with a
host-side node permutation that bin-packs nodes into 128-dst windows by
in-degree so every (core, window) has ~equal edge count. Per conv layer
each core computes its msg shard (node-wise Fourier-KAN) in bf16 packed
2-nodes-per-256B-row (node pair duplicated to satisfy the 256B gather
granularity), an AllGather builds the full msg table in DRAM, then async
dma_gather segments (rotating the 4 SWDGE queues so their drains overlap)
pull per-edge source rows. Host-precomputed one-hot matrices (DMA'd from
DRAM, no on-device IS_EQ) drive bf16 scatter-matmuls accumulating each
128-dst window in PSUM. The next layer's msg compute is emitted per
32-feature block as soon as that block's windows finish, hiding it under
the scatter. Pool via one-hot matmul + small AllReduce; readout + sigmoid
on device.

The Fourier features sin/cos(k*h), k=1..4 are built from sin(h), cos(h)
(range-reduced via round-to-nearest f32->i32 cast) plus ScalarE Square
chains; the k-harmonics are linear in 8 basis tensors, so the KAN weights
are remixed host-side onto that basis (plus a per-output bias column).
"""

import heapq
import math
import numpy as np
import ml_dtypes

import concourse.bacc as bacc
import concourse.mybir as mybir
import concourse.tile as tile
from concourse.bass_utils import run_bass_kernel_spmd

F32 = mybir.dt.float32
BF16 = mybir.dt.bfloat16
I16 = mybir.dt.int16
I32 = mybir.dt.int32
F16 = mybir.dt.float16
AF = mybir.ActivationFunctionType
OP = mybir.AluOpType

P = 8
HID = 32
INF = 64
NG = 128
NCONV = 2
NEG = 0.01

NPC = 6656                 # nodes per core (padded total 53248)
NTOT = NPC * P
NBLK = 4
BLK = NPC // NBLK          # 1664
WIN = 128
NWIN = NPC // WIN          # 52
N_NODES_REAL = 50000
NROWS = NTOT // 2          # 2 nodes per 256B bf16 row (pair duplicated)
SEGC = 8                   # chunks per gather segment (1024 tokens)

TWO_PI = float(2 * math.pi)
PI = float(math.pi)
INV_2PI = float(1.0 / (2 * math.pi))
ISQ2 = float(1.0 / math.sqrt(2.0))

LAST_RESULTS = None        # test.py reads exec_time_ns from here


def _install_ntff_hook():
    # restore the axon NTFF profiling hook when the image's antenv lacks it
    import sys
    import types
    try:
        import antenv.axon_hooks  # noqa: F401
        return
    except ImportError:
        pass
    try:
        import antenv
        from trn_agent_boot.trn_boot import _ntff_profile_via_ctypes
        hook = _ntff_profile_via_ctypes("/opt/axon/libaxon_pjrt.so")
        mod = types.ModuleType("antenv.axon_hooks")
        holder = {"h": hook}
        mod.set_axon_ntff_profile_hook = lambda h: holder.__setitem__("h", h)
        mod.get_axon_ntff_profile_hook = lambda: holder["h"]
        sys.modules["antenv.axon_hooks"] = mod
        antenv.axon_hooks = mod
    except Exception:
        pass


_install_ntff_hook()


# ----------------------------------------------------------------------------
# host-side sharding / index prep
# ----------------------------------------------------------------------------

def _balance_nodes(dst):
    """Bin-pack real nodes into P*NWIN windows of <=128 so every window's
    in-edge count is ~equal. Returns perm: old node id -> new node id."""
    indeg = np.bincount(dst, minlength=N_NODES_REAL).astype(np.int64)
    order = np.argsort(-indeg, kind="stable")
    nw = P * NWIN
    heap = [(0, 0, w) for w in range(nw)]   # (sum, count, window)
    heapq.heapify(heap)
    wslot = np.empty(N_NODES_REAL, dtype=np.int64)
    wcnt = np.zeros(nw, dtype=np.int64)
    for i in order:
        s, c, w = heapq.heappop(heap)
        wslot[i] = w
        cpos = wcnt[w]
        wcnt[w] += 1
        if c + 1 < WIN:
            heapq.heappush(heap, (s + indeg[i], c + 1, w))
        # record position later via stable counting
    # positions: stable order of assignment per window
    perm = np.empty(N_NODES_REAL, dtype=np.int64)
    pos_in_w = np.zeros(nw, dtype=np.int64)
    for i in order:
        w = wslot[i]
        core, wl = w // NWIN, w % NWIN
        perm[i] = core * NPC + wl * WIN + pos_in_w[w]
        pos_in_w[w] += 1
    return perm


def _prep(edge_index, batch):
    src0 = np.asarray(edge_index[0], dtype=np.int64)
    dst0 = np.asarray(edge_index[1], dtype=np.int64)
    bat0 = np.asarray(batch, dtype=np.int64)
    E = src0.shape[0]

    perm = _balance_nodes(dst0)
    src = perm[src0]
    dst = perm[dst0]

    core = dst // NPC
    w_in_core = (dst % NPC) // WIN
    dloc = dst % WIN
    j = src % 2                               # parity: 32-col slice of row
    row = src // 2                            # table row (node pair, dup'd)

    key = (core * NWIN + w_in_core) * 2 + j
    cnt = np.bincount(key, minlength=P * NWIN * 2).reshape(P, NWIN, 2)
    cmax = cnt.max(axis=0)                    # (NWIN, 2)
    nch = np.ceil(cmax / 128).astype(np.int64)
    nch = np.maximum(nch, (cmax > 0))

    # chunk layout per window: parity-0 chunks then parity-1 chunks
    chunk_base = np.zeros((NWIN, 2), dtype=np.int64)
    win_plan = []                             # per window: [(chunk, j), ...]
    ch = 0
    for w in range(NWIN):
        entries = []
        for jj in range(2):
            chunk_base[w, jj] = ch
            for _ in range(int(nch[w, jj])):
                entries.append((ch, jj))
                ch += 1
        win_plan.append(entries)
    CH = ch
    NSEG = (CH + SEGC - 1) // SEGC
    CHP = NSEG * SEGC
    NTOKP = CHP * 128

    # sort edges by (core, window, parity, row) -> slots
    skey = key * (NROWS + 1) + row
    order = np.argsort(skey, kind="stable")
    s_key = key[order]
    s_row = row[order]
    s_dloc = dloc[order]
    grp_start = np.zeros(P * NWIN * 2, dtype=np.int64)
    grp_start[1:] = np.cumsum(np.bincount(s_key, minlength=P * NWIN * 2))[:-1]
    pos = np.arange(E) - grp_start[s_key]

    s_core = s_key // (NWIN * 2)
    s_w = (s_key // 2) % NWIN
    s_j = s_key % 2
    slot = s_core * NTOKP + chunk_base[s_w, s_j] * 128 + pos

    tok_row = ((np.arange(P * NTOKP) * 9973) % NROWS).astype(np.int16)
    tok_row[slot] = s_row.astype(np.int16)

    ti = tok_row.reshape(P, NTOKP // 16, 16)
    ti = np.swapaxes(ti, 1, 2)                               # (P, 16, cols)
    gidx_dev = np.tile(ti, (1, 8, 1)).copy()                 # (P, 128, cols)

    # per-token dst-local ids (255 = dummy -> zero one-hot row on device)
    tok_dloc = np.full(P * NTOKP, 255.0, dtype=np.float32)
    tok_dloc[slot] = s_dloc.astype(np.float32)
    td = tok_dloc.reshape(P, CHP, 128)
    dloc_dev = np.ascontiguousarray(
        np.swapaxes(td, 1, 2)).astype(ml_dtypes.bfloat16)    # (P, 128, CHP)

    bat = np.full(N_NODES_REAL, 0, dtype=np.int64)
    bat[:] = bat0
    bat_pad = np.full(NTOT, -1, dtype=np.int64)
    bat_pad[perm] = bat
    B = (bat_pad[:, None] == np.arange(NG)[None, :])
    B_dev = np.ascontiguousarray(
        B.reshape(P, NWIN, 128, NG).transpose(0, 2, 1, 3)
        .reshape(P, 128, NWIN * NG)).astype(ml_dtypes.bfloat16)
    counts = np.bincount(bat0, minlength=NG)[:NG].astype(np.float32)
    invc = (1.0 / np.maximum(counts, 1.0)).reshape(NG, 1)

    return dict(CH=CH, CHP=CHP, NSEG=NSEG, win_plan=win_plan, perm=perm,
                gidx_dev=gidx_dev, dloc_dev=dloc_dev, B_dev=B_dev, invc=invc)


def _pack_x(x):
    xp = np.zeros((NTOT, INF), dtype=np.float32)
    xp[:x.shape[0]] = x
    xc = xp.reshape(P, 2, NPC // 2, INF)
    return np.ascontiguousarray(xc.transpose(0, 1, 3, 2).reshape(P, 128, NPC // 2))


def _remix(W):
    """W: (2, out, in, 4) -> 8 slot matrices (out, in) + bias (out,).

    Basis slots: [sin h, cos h, sin^2 h, (1+sin2h)/2, sin^2 2h,
                  (1-sin4h)/2, sin3h, cos3h]."""
    W0, W1 = W[0], W[1]          # cos / sin coefficient stacks
    slots = [
        W1[:, :, 0],
        W0[:, :, 0],
        -2.0 * W0[:, :, 1],
        2.0 * W1[:, :, 1],
        -2.0 * W0[:, :, 3],
        -2.0 * W1[:, :, 3],
        W1[:, :, 2],
        W0[:, :, 2],
    ]
    bias = (W0[:, :, 1] - W1[:, :, 1] + W0[:, :, 3] + W1[:, :, 3]).sum(axis=1)
    return slots, bias.astype(np.float32)


def _pack_weights(W_in, W_conv, W_out):
    sl_in, b_in = _remix(W_in)
    # input: fused stationary per (half, slot): 128x128 with 64x32 blocks at
    # (rows 0:64 -> out 32*half) and (rows 64:128 -> out 32*(half+2))
    win = np.zeros((128, 2 * 8 * 128), dtype=np.float16)
    for h in range(2):
        for m in range(8):
            c0 = (h * 8 + m) * 128
            win[0:64, c0 + 32 * h:c0 + 32 * h + 32] = (
                sl_in[m].T.astype(np.float16))
            win[64:128, c0 + 32 * (h + 2):c0 + 32 * (h + 2) + 32] = (
                sl_in[m].T.astype(np.float16))
    # conv: fused stationary per (layer, slot): block-diag of 4 identical
    # 32x32 tiles
    wc = np.zeros((128, NCONV * 8 * 128), dtype=np.float16)
    biases = np.zeros((128, 1 + NCONV), dtype=np.float32)
    biases[:, 0] = np.tile(b_in, 4)
    for l in range(NCONV):
        sl, bl = _remix(W_conv[l])
        biases[:, 1 + l] = np.tile(bl, 4)
        for b in range(4):
            for m in range(8):
                c0 = (l * 8 + m) * 128
                wc[32 * b:32 * b + 32, c0 + 32 * b:c0 + 32 * b + 32] = (
                    sl[m].T.astype(np.float16))
    w0r = np.tile(W_out[0, 0, :, 0].astype(np.float32), (128, 1))
    w1r = np.tile(W_out[1, 0, :, 0].astype(np.float32), (128, 1))
    return win, wc, biases, w0r, w1r


# ----------------------------------------------------------------------------
# device program
# ----------------------------------------------------------------------------

def _build(meta):
    CH, CHP, NSEG = meta["CH"], meta["CHP"], meta["NSEG"]
    win_plan = meta["win_plan"]
    XCOLS = NPC // 2                  # 3328
    NTOKP = CHP * 128

    nc = bacc.Bacc("TRN2", target_bir_lowering=False, debug=False,
                   num_devices=P, num_swdge_queues=4)

    x_d = nc.dram_tensor("x_pack", [128, XCOLS], F32, kind="ExternalInput")
    win_d = nc.dram_tensor("win_w", [128, 2 * 8 * 128], F16, kind="ExternalInput")
    wc_d = nc.dram_tensor("wc_w", [128, NCONV * 8 * 128], F16, kind="ExternalInput")
    bias_d = nc.dram_tensor("biases", [128, 1 + NCONV], F32, kind="ExternalInput")
    w0_d = nc.dram_tensor("w0r", [128, HID], F32, kind="ExternalInput")
    w1_d = nc.dram_tensor("w1r", [128, HID], F32, kind="ExternalInput")
    bout_d = nc.dram_tensor("bout", [128, 1], F32, kind="ExternalInput")
    invc_d = nc.dram_tensor("invc", [128, 1], F32, kind="ExternalInput")
    gidx_d = nc.dram_tensor("gidx", [128, NTOKP // 16], I16, kind="ExternalInput")
    dloc_d = nc.dram_tensor("dloc", [128, CHP], BF16, kind="ExternalInput")
    iota_d = nc.dram_tensor("iota", [128, 128], BF16, kind="ExternalInput")
    id32_d = nc.dram_tensor("id32", [128, 32], F32, kind="ExternalInput")
    id32b_d = nc.dram_tensor("id32b", [128, 32], BF16, kind="ExternalInput")
    B_d = nc.dram_tensor("Bmat", [128, NWIN * NG], BF16, kind="ExternalInput")

    out_d = nc.dram_tensor("out", [NG, 1], F32, kind="ExternalOutput")

    AG_GROUPS = [list(range(P))]
    NT = 416
    GBUF = 11

    with tile.TileContext(nc) as tc:
        with (
            tc.tile_pool(name="const", bufs=1) as cp,
            tc.tile_pool(name="feat", bufs=1) as fp,
            tc.tile_pool(name="ftmp", bufs=1) as tp,
            tc.tile_pool(name="work", bufs=1) as wp,
            tc.tile_pool(name="gbuf", bufs=1) as gp,
            tc.tile_pool(name="ohp", bufs=1) as ohp,
            tc.tile_pool(name="pmsg", bufs=2, space="PSUM") as pmsg_p,
            tc.tile_pool(name="ptr", bufs=2, space="PSUM") as ptr_p,
            tc.tile_pool(name="pm", bufs=3, space="PSUM") as pm_p,
            tc.tile_pool(name="ppool", bufs=1, space="PSUM") as ppool_p,
            tc.tile_pool(name="dram", bufs=1, space="DRAM") as dp,
        ):
            # ---- constants ----
            x_sb = wp.tile([128, XCOLS], F32, name="x_sb", tag="bigx")
            nc.sync.dma_start(x_sb[:], x_d[:])
            win_sb = cp.tile([128, 2 * 8 * 128], F16)
            nc.sync.dma_start(win_sb[:], win_d[:])
            wc_sb = cp.tile([128, NCONV * 8 * 128], F16)
            nc.sync.dma_start(wc_sb[:], wc_d[:])
            bias_sb = cp.tile([128, 1 + NCONV], F32)
            nc.sync.dma_start(bias_sb[:], bias_d[:])
            w0_sb = cp.tile([128, HID], F32)
            nc.sync.dma_start(w0_sb[:], w0_d[:])
            w1_sb = cp.tile([128, HID], F32)
            nc.sync.dma_start(w1_sb[:], w1_d[:])
            bout_sb = cp.tile([128, 1], F32)
            nc.sync.dma_start(bout_sb[:], bout_d[:])
            invc_sb = cp.tile([128, 1], F32)
            nc.sync.dma_start(invc_sb[:], invc_d[:])
            gidx_sb = cp.tile([128, NTOKP // 16], I16)
            nc.sync.dma_start(gidx_sb[:], gidx_d[:])
            dloc_sb = cp.tile([128, CHP], BF16)
            nc.sync.dma_start(dloc_sb[:], dloc_d[:])
            iota_sb = cp.tile([128, 128], BF16)
            nc.sync.dma_start(iota_sb[:], iota_d[:])
            id32_sb = cp.tile([128, 32], F32)
            nc.sync.dma_start(id32_sb[:], id32_d[:])
            id32b_sb = cp.tile([128, 32], BF16)
            nc.sync.dma_start(id32b_sb[:], id32b_d[:])
            zb = cp.tile([128, 1], F32)
            nc.vector.memset(zb[:], 0.0)
            m1 = cp.tile([128, 1], F32)
            nc.vector.memset(m1[:], -1.0)

            h_sb = cp.tile([128, BLK], F32)    # packed h^T: partition 32*blk+f

            shard = [dp.tile([NPC // 2, 128], BF16, name=f"shard{l}")
                     for l in range(NCONV)]
            table = [dp.tile([NROWS, 128], BF16, name=f"table{l}",
                             addr_space="Shared")
                     for l in range(NCONV)]
            pool_in = dp.tile([HID, NG], F32)
            pool_out = dp.tile([HID, NG], F32, addr_space="Shared")

            def feat_chain(src, FREE, pfx, row0, nrows):
                """8 f16 basis-feature tiles (rows row0:row0+nrows valid)."""
                ps = slice(row0, row0 + nrows)

                def ts(dst, a, s1, s2, o0, o1=None):
                    if o1 is None:
                        nc.vector.tensor_scalar(dst, a, s1, None, o0)
                    else:
                        nc.vector.tensor_scalar(dst, a, s1, s2, o0, o1)

                def scr(nm, dt=F32):
                    t = tp.tile([128, FREE], dt, name=f"{pfx}{nm}", tag="scr",
                                bufs=3, padded_shape=[128, BLK])
                    return t[ps, :]

                def keep(nm):
                    t = tp.tile([128, FREE], F32, name=f"{pfx}{nm}", tag=nm,
                                bufs=1, padded_shape=[128, BLK])
                    return t[ps, :]

                slots = [fp.tile([128, FREE], F16, name=f"{pfx}slot{i}",
                                 tag=f"feat{i}", padded_shape=[128, BLK])
                         for i in range(8)]
                sl = [s[ps, :] for s in slots]
                zbs, m1s = zb[ps, :], m1[ps, :]

                n0 = scr("n0", I32)
                ts(n0, src, INV_2PI, None, OP.mult)
                nf0 = scr("nf0")
                ts(nf0, n0, -TWO_PI, None, OP.mult)
                r0 = scr("r0")
                nc.vector.tensor_tensor(r0, src, nf0, OP.add)
                n9 = scr("n9", I32)
                ts(n9, src, INV_2PI, 0.25, OP.mult, OP.add)
                nf9 = scr("nf9")
                ts(nf9, n9, -TWO_PI, PI / 2, OP.mult, OP.add)
                r9 = scr("r9")
                nc.vector.tensor_tensor(r9, src, nf9, OP.add)

                s1f = keep("s1f")
                nc.scalar.activation(s1f, r0, AF.Sin, bias=zbs)
                c1f = keep("c1f")
                nc.scalar.activation(c1f, r9, AF.Sin, bias=zbs)
                nc.scalar.activation(sl[0], r0, AF.Sin, bias=zbs)
                nc.scalar.activation(sl[1], r9, AF.Sin, bias=zbs)
                sqsf = keep("sqsf")
                nc.scalar.activation(sqsf, s1f, AF.Square)
                nc.scalar.activation(sl[2], s1f, AF.Square)
                d = scr("d")
                nc.vector.tensor_tensor(d, s1f, c1f, OP.add)
                sqdf = keep("sqdf")
                nc.scalar.activation(sqdf, d, AF.Square, scale=ISQ2)
                nc.scalar.activation(sl[3], d, AF.Square, scale=ISQ2)
                nc.scalar.activation(sl[4], sqdf, AF.Square,
                                     bias=m1s, scale=2.0)
                tc2 = scr("tc2")
                ts(tc2, sqsf, -2.0, 1.0, OP.mult, OP.add)
                ts2 = scr("ts2")
                ts(ts2, sqdf, 2.0, -1.0, OP.mult, OP.add)
                td2 = scr("td2")
                nc.vector.tensor_tensor(td2, tc2, ts2, OP.subtract)
                nc.scalar.activation(sl[5], td2, AF.Square, scale=ISQ2)
                t3 = scr("t3")
                ts(t3, sqsf, -4.0, 3.0, OP.mult, OP.add)
                nc.vector.tensor_tensor(sl[6], s1f, t3, OP.mult)
                t4 = scr("t4")
                ts(t4, sqsf, -4.0, 1.0, OP.mult, OP.add)
                nc.vector.tensor_tensor(sl[7], c1f, t4, OP.mult)
                return slots

            def msg_full(l):
                """Emit msg compute for conv layer l (full width)."""
                mTp = wp.tile([128, BLK], BF16, name=f"mT{l}",
                              tag="mT", bufs=2)
                slots = feat_chain(h_sb[:], BLK, f"m{l}_", 0, 128)
                msg_matmuls(l, slots, mTp, 0)
                msg_stage(l, mTp)

            def msg_matmuls(l, slots, mTp, c0):
                for t4i in range(4):
                    pm2 = pmsg_p.tile([128, NT], F32, name="pm2", tag="pmsg")
                    for m in range(8):
                        nc.tensor.matmul(
                            pm2[:],
                            wc_sb[:, (l * 8 + m) * 128:(l * 8 + m + 1) * 128],
                            slots[m][:, NT * t4i:NT * (t4i + 1)],
                            start=(m == 0), stop=(m == 7),
                        )
                    nc.vector.tensor_scalar(
                        mTp[:, c0 + NT * t4i:c0 + NT * (t4i + 1)], pm2[:],
                        bias_sb[:, 1 + l:2 + l], None, OP.add)

            def msg_stage(l, mTp):
                # reorder columns: even nodes first, then odd, per w2 tile
                mTe = wp.tile([128, BLK], BF16, name=f"mTe{l}",
                              tag="mTe", bufs=2)
                nc.vector.tensor_copy(
                    mTe[:].rearrange("p (w2 sub rh) -> p w2 sub rh",
                                     w2=13, sub=2),
                    mTp[:].rearrange("p (w2 rh sub) -> p w2 sub rh",
                                     w2=13, sub=2))
                for b in range(NBLK):
                    ps = slice(32 * b, 32 * b + 32)
                    stage = wp.tile([128, 13 * HID], BF16, name=f"stg{l}_{b}",
                                    tag="stage", bufs=2)
                    for w2 in range(13):
                        ptr = ptr_p.tile([128, 32], BF16, name="ptrt", tag="ptr")
                        nc.tensor.transpose(
                            ptr[:], mTe[ps, 128 * w2:128 * (w2 + 1)],
                            id32b_sb[ps, :],
                            tile_position=(32 * b, 0))
                        nc.scalar.activation(stage[:, 32 * w2:32 * (w2 + 1)],
                                             ptr[:], AF.Copy)
                    # partitions 0:64 = even rows, 64:128 = odd rows
                    for sub in range(2):
                        nc.sync.dma_start(
                            shard[l][832 * b:832 * (b + 1),
                                     32 * sub:32 * sub + 32].rearrange(
                                "(w2 rh) f -> rh w2 f", w2=13),
                            stage[64 * sub:64 * sub + 64, :].rearrange(
                                "p (w2 f) -> p w2 f", f=HID),
                        )

            # ================= input KAN: x -> h =================
            _s_in = nc.named_scope("ph_input"); _s_in.__enter__()
            for half in range(2):
                xsl = x_sb[:, half * BLK:(half + 1) * BLK]
                slots = feat_chain(xsl, BLK, f"x{half}_", 0, 128)
                for t4i in range(BLK // NT):
                    off = NT * t4i
                    ph = pmsg_p.tile([128, NT], F32, name="ph", tag="pmsg")
                    for m in range(8):
                        nc.tensor.matmul(
                            ph[:],
                            win_sb[:, (half * 8 + m) * 128:(half * 8 + m + 1) * 128],
                            slots[m][:, NT * t4i:NT * (t4i + 1)],
                            start=(m == 0), stop=(m == 7),
                        )
                    for hb in (half, half + 2):
                        nc.vector.tensor_scalar(
                            h_sb[32 * hb:32 * hb + 32, off:off + NT],
                            ph[32 * hb:32 * hb + 32, :],
                            bias_sb[32 * hb:32 * hb + 32, 0:1], None, OP.add)
            msg_full(0)
            _s_in.__exit__(None, None, None)

            # pool resources (consumed per-block during the last scatter)
            B_sb = wp.tile([128, NWIN * NG], BF16, name="B_sb", tag="bigx")
            nc.sync.dma_start(B_sb[:], B_d[:])
            ppool = ppool_p.tile([HID, NG], F32)
            hbf = wp.tile([128, BLK], BF16, name="hbf", tag="hbf", bufs=1)

            def pool_block(b):
                ps = slice(32 * b, 32 * b + 32)
                nc.vector.tensor_copy(hbf[ps, :], h_sb[ps, :])
                for w2 in range(13):
                    w = 13 * b + w2
                    ptb = ptr_p.tile([128, 32], BF16, name="ptb", tag="ptr")
                    nc.tensor.transpose(
                        ptb[:], hbf[ps, 128 * w2:128 * (w2 + 1)],
                        id32b_sb[ps, :], tile_position=(32 * b, 0))
                    htile = wp.tile([128, 32], BF16, name="htile",
                                    tag="htile", bufs=3)
                    nc.scalar.activation(htile[:], ptb[:], AF.Copy)
                    nc.tensor.matmul(
                        ppool[:], htile[:], B_sb[:, NG * w:NG * (w + 1)],
                        start=(w == 0), stop=(w == NWIN - 1),
                    )

            _s_ag = nc.named_scope("ph_ag0"); _s_ag.__enter__()
            nc.gpsimd.collective_compute(
                "AllGather", OP.bypass,
                ins=[shard[0][:]], outs=[table[0][:]],
                replica_groups=AG_GROUPS,
            )
            _s_ag.__exit__(None, None, None)

            # ================= conv layers: scatter (+ pipelined msg) ======
            for l in range(NCONV):
                _s_sc = nc.named_scope(f"ph_scat{l}"); _s_sc.__enter__()
                Gs = [None] * NSEG
                OHs = [None] * NSEG
                state = {"issued": 0, "built": 0}

                def issue_seg(s, l=l, Gs=Gs):
                    G = gp.tile([128, SEGC, 128], BF16, name=f"G{l}_{s}",
                                tag=f"G{s % GBUF}")
                    nc.gpsimd.dma_gather(
                        G[:], table[l][:],
                        gidx_sb[:, s * (SEGC * 8):(s + 1) * (SEGC * 8)],
                        num_idxs=SEGC * 128, num_idxs_reg=SEGC * 128,
                        elem_size=128, single_packet=False, queue_num=s % 4,
                    )
                    Gs[s] = G

                iota_b = iota_sb[:].rearrange("p (x d) -> p x d", x=1)

                def load_oh(s, l=l, OHs=OHs):
                    oh = ohp.tile([128, SEGC, 128], BF16, name=f"oh{l}_{s}",
                                  tag=f"oh{s % 4}")
                    for hseg in range(2):
                        c0 = s * SEGC + hseg * (SEGC // 2)
                        nc.vector.tensor_tensor(
                            oh[:, hseg * (SEGC // 2):(hseg + 1) * (SEGC // 2), :],
                            iota_b.to_broadcast([128, SEGC // 2, 128]),
                            dloc_sb[:, c0:c0 + SEGC // 2]
                            .to_broadcast([128, SEGC // 2, 128]),
                            OP.is_equal)
                    OHs[s] = oh

                for w in range(NWIN):
                    entries = win_plan[w]
                    if entries:
                        cg = entries[0][0]
                        g_need = min(cg // SEGC + (GBUF - 1), NSEG - 1)
                        while state["issued"] <= g_need:
                            issue_seg(state["issued"])
                            state["issued"] += 1
                        oh_need = min((cg + len(entries) - 1) // SEGC + 1,
                                      NSEG - 1)
                        while state["built"] <= oh_need:
                            load_oh(state["built"])
                            state["built"] += 1
                        hb, off = (w * WIN) // BLK, (w * WIN) % BLK
                        pm = pm_p.tile([128, WIN], F32, name="pmw", tag="pm")
                        pms = pm[32 * hb:32 * hb + 32, :]
                        nent = len(entries)
                        for i, (c, jj) in enumerate(entries):
                            s, cl = c // SEGC, c % SEGC
                            nc.tensor.matmul(
                                pms, Gs[s][:, cl, 32 * jj:32 * jj + 32],
                                OHs[s][:, cl, :],
                                start=(i == 0), stop=(i == nent - 1),
                                tile_position=(0, 32 * hb),
                            )
                        hsl = h_sb[32 * hb:32 * hb + 32, off:off + WIN]
                        nc.vector.tensor_tensor(hsl, pms, hsl, OP.add)
                # leaky relu: h = max(z, 0.01*z)
                lrt = wp.tile([128, BLK], F32, name=f"lr{l}",
                              tag="lrt", bufs=2)
                nc.vector.tensor_scalar(lrt[:], h_sb[:], NEG, None, OP.mult)
                nc.vector.tensor_tensor(h_sb[:], h_sb[:], lrt[:], OP.max)
                if l + 1 < NCONV:
                    msg_full(l + 1)
                else:
                    for b in range(NBLK):
                        pool_block(b)
                _s_sc.__exit__(None, None, None)
                if l + 1 < NCONV:
                    _s_ag1 = nc.named_scope(f"ph_ag{l+1}"); _s_ag1.__enter__()
                    nc.gpsimd.collective_compute(
                        "AllGather", OP.bypass,
                        ins=[shard[l + 1][:]], outs=[table[l + 1][:]],
                        replica_groups=AG_GROUPS,
                    )
                    _s_ag1.__exit__(None, None, None)

            # ================= pool + readout =================
            _s_po = nc.named_scope("ph_pool"); _s_po.__enter__()
            pool_sb = wp.tile([HID, NG], F32, name="pool_sb")
            nc.vector.tensor_copy(pool_sb[:], ppool[:])
            nc.sync.dma_start(pool_in[:], pool_sb[:])
            nc.gpsimd.collective_compute(
                "AllReduce", OP.add,
                ins=[pool_in[:]], outs=[pool_out[:]],
                replica_groups=AG_GROUPS,
            )
            psum_sb = wp.tile([HID, NG], F32, name="psum_sb")
            nc.sync.dma_start(psum_sb[:], pool_out[:])
            ptry = ptr_p.tile([128, 32], F32, name="ptry", tag="ptr")
            nc.tensor.transpose(ptry[:], psum_sb[:], id32_sb[0:32, :])
            y_sb = wp.tile([NG, HID], F32, name="y_sb")
            nc.vector.tensor_scalar(y_sb[:], ptry[:], invc_sb[:], None, OP.mult)

            # readout: sin(y), cos(y) via the same range reduction
            def sincos(src, pfx, quarter):
                n = wp.tile([NG, HID], I32, name=f"{pfx}n")
                nf = wp.tile([NG, HID], F32, name=f"{pfx}nf")
                if quarter:
                    nc.vector.tensor_scalar(n[:], src, INV_2PI, 0.25, OP.mult, OP.add)
                    nc.vector.tensor_scalar(nf[:], n[:], -TWO_PI, PI / 2,
                                            OP.mult, OP.add)
                else:
                    nc.vector.tensor_scalar(n[:], src, INV_2PI, None, OP.mult)
                    nc.vector.tensor_scalar(nf[:], n[:], -TWO_PI, None, OP.mult)
                r = wp.tile([NG, HID], F32, name=f"{pfx}r")
                nc.vector.tensor_tensor(r[:], src, nf[:], OP.add)
                o = wp.tile([NG, HID], F32, name=f"{pfx}o")
                nc.scalar.activation(o[:], r[:], AF.Sin, bias=zb[:])
                return o

            sin_y = sincos(y_sb[:], "sy", False)
            cos_y = sincos(y_sb[:], "cy", True)
            nc.vector.tensor_tensor(cos_y[:], cos_y[:], w0_sb[:], OP.mult)
            nc.vector.tensor_tensor(sin_y[:], sin_y[:], w1_sb[:], OP.mult)
            nc.vector.tensor_tensor(cos_y[:], cos_y[:], sin_y[:], OP.add)
            red = wp.tile([NG, 1], F32, name="red")
            nc.vector.tensor_reduce(red[:], cos_y[:], mybir.AxisListType.X, OP.add)
            o_sb = wp.tile([NG, 1], F32, name="o_sb")
            nc.scalar.activation(o_sb[:], red[:], AF.Sigmoid, bias=bout_sb[:])
            nc.sync.dma_start(out_d[:], o_sb[:])
            _s_po.__exit__(None, None, None)

    nc.compile()
    return nc


# ----------------------------------------------------------------------------
# entry point
# ----------------------------------------------------------------------------

def kernel(x, edge_index, batch, W_in, W_conv, W_out, b_out):
    global LAST_RESULTS
    x = np.asarray(x, dtype=np.float32)
    W_in = np.asarray(W_in, dtype=np.float32)
    W_conv = np.asarray(W_conv, dtype=np.float32)
    W_out = np.asarray(W_out, dtype=np.float32)
    b_out = np.asarray(b_out, dtype=np.float32)

    meta = _prep(edge_index, batch)
    perm = meta["perm"]
    x_perm = np.zeros((NTOT, INF), dtype=np.float32)
    x_perm[perm] = x
    x_pack = _pack_x(x_perm)
    win, wc, biases, w0r, w1r = _pack_weights(W_in, W_conv, W_out)

    nc = _build(meta)

    iota = np.tile(np.arange(128, dtype=np.float32)[None, :],
                   (128, 1)).astype(ml_dtypes.bfloat16)
    id32 = np.tile(np.eye(32, dtype=np.float32), (4, 1))
    id32b = np.tile(np.eye(32, dtype=ml_dtypes.bfloat16), (4, 1))
    bout_col = np.full((128, 1), float(b_out.ravel()[0]), dtype=np.float32)

    in_maps = []
    for c in range(P):
        in_maps.append({
            "x_pack": x_pack[c],
            "win_w": win,
            "wc_w": wc,
            "biases": biases,
            "w0r": w0r,
            "w1r": w1r,
            "bout": bout_col,
            "invc": meta["invc"].astype(np.float32),
            "gidx": meta["gidx_dev"][c],
            "dloc": meta["dloc_dev"][c],
            "iota": iota,
            "id32": id32,
            "id32b": id32b,
            "Bmat": meta["B_dev"][c],
        })

    import os as _os
    _tc = _os.environ.get("TRACE_CORES")
    _kw = {}
    if _tc:
        _kw = dict(trace_cores=[int(x) for x in _tc.split(",")], stitch_traces=True)
    res = run_bass_kernel_spmd(nc, in_maps, core_ids=list(range(P)), **_kw)
    LAST_RESULTS = res
    return np.asarray(res.results[0]["out"], dtype=np.float32)


# revision 26
# speedup vs baseline: 1.0154x; 1.0154x over previous
"""KA-GNN (Fourier-KAN message passing) on 8 Trainium2 NeuronCores — v3.

Sharding: nodes/edges partitioned by destination across 8 cores, with a
host-side node permutation that bin-packs nodes into 128-dst windows by
in-degree so every (core, window) has ~equal edge count. Per conv layer
each core computes its msg shard (node-wise Fourier-KAN) in bf16 packed
2-nodes-per-256B-row (node pair duplicated to satisfy the 256B gather
granularity), an AllGather builds the full msg table in DRAM, then async
dma_gather segments (rotating the 4 SWDGE queues so their drains overlap)
pull per-edge source rows. Host-precomputed one-hot matrices (DMA'd from
DRAM, no on-device IS_EQ) drive bf16 scatter-matmuls accumulating each
128-dst window in PSUM. The next layer's msg compute is emitted per
32-feature block as soon as that block's windows finish, hiding it under
the scatter. Pool via one-hot matmul + small AllReduce; readout + sigmoid
on device.

The Fourier features sin/cos(k*h), k=1..4 are built from sin(h), cos(h)
(range-reduced via round-to-nearest f32->i32 cast) plus ScalarE Square
chains; the k-harmonics are linear in 8 basis tensors, so the KAN weights
are remixed host-side onto that basis (plus a per-output bias column).
"""

import heapq
import math
import numpy as np
import ml_dtypes

import concourse.bacc as bacc
import concourse.mybir as mybir
import concourse.tile as tile
from concourse.bass_utils import run_bass_kernel_spmd

F32 = mybir.dt.float32
BF16 = mybir.dt.bfloat16
I16 = mybir.dt.int16
I32 = mybir.dt.int32
F16 = mybir.dt.float16
AF = mybir.ActivationFunctionType
OP = mybir.AluOpType

P = 8
HID = 32
INF = 64
NG = 128
NCONV = 2
NEG = 0.01

NPC = 6656                 # nodes per core (padded total 53248)
NTOT = NPC * P
NBLK = 4
BLK = NPC // NBLK          # 1664
WIN = 128
NWIN = NPC // WIN          # 52
N_NODES_REAL = 50000
NROWS = NTOT // 2          # 2 nodes per 256B bf16 row (pair duplicated)
SEGC = 16                  # chunks per gather segment (2048 tokens)

TWO_PI = float(2 * math.pi)
PI = float(math.pi)
INV_2PI = float(1.0 / (2 * math.pi))
ISQ2 = float(1.0 / math.sqrt(2.0))

LAST_RESULTS = None        # test.py reads exec_time_ns from here


def _install_ntff_hook():
    # restore the axon NTFF profiling hook when the image's antenv lacks it
    import sys
    import types
    try:
        import antenv.axon_hooks  # noqa: F401
        return
    except ImportError:
        pass
    try:
        import antenv
        from trn_agent_boot.trn_boot import _ntff_profile_via_ctypes
        hook = _ntff_profile_via_ctypes("/opt/axon/libaxon_pjrt.so")
        mod = types.ModuleType("antenv.axon_hooks")
        holder = {"h": hook}
        mod.set_axon_ntff_profile_hook = lambda h: holder.__setitem__("h", h)
        mod.get_axon_ntff_profile_hook = lambda: holder["h"]
        sys.modules["antenv.axon_hooks"] = mod
        antenv.axon_hooks = mod
    except Exception:
        pass


_install_ntff_hook()


# ----------------------------------------------------------------------------
# host-side sharding / index prep
# ----------------------------------------------------------------------------

def _balance_nodes(dst):
    """Bin-pack real nodes into P*NWIN windows of <=128 so every window's
    in-edge count is ~equal. Returns perm: old node id -> new node id."""
    indeg = np.bincount(dst, minlength=N_NODES_REAL).astype(np.int64)
    order = np.argsort(-indeg, kind="stable")
    nw = P * NWIN
    heap = [(0, 0, w) for w in range(nw)]   # (sum, count, window)
    heapq.heapify(heap)
    wslot = np.empty(N_NODES_REAL, dtype=np.int64)
    wcnt = np.zeros(nw, dtype=np.int64)
    for i in order:
        s, c, w = heapq.heappop(heap)
        wslot[i] = w
        cpos = wcnt[w]
        wcnt[w] += 1
        if c + 1 < WIN:
            heapq.heappush(heap, (s + indeg[i], c + 1, w))
        # record position later via stable counting
    # positions: stable order of assignment per window
    perm = np.empty(N_NODES_REAL, dtype=np.int64)
    pos_in_w = np.zeros(nw, dtype=np.int64)
    for i in order:
        w = wslot[i]
        core, wl = w // NWIN, w % NWIN
        perm[i] = core * NPC + wl * WIN + pos_in_w[w]
        pos_in_w[w] += 1
    return perm


def _prep(edge_index, batch):
    src0 = np.asarray(edge_index[0], dtype=np.int64)
    dst0 = np.asarray(edge_index[1], dtype=np.int64)
    bat0 = np.asarray(batch, dtype=np.int64)
    E = src0.shape[0]

    perm = _balance_nodes(dst0)
    src = perm[src0]
    dst = perm[dst0]

    core = dst // NPC
    w_in_core = (dst % NPC) // WIN
    dloc = dst % WIN
    j = src % 2                               # parity: 32-col slice of row
    row = src // 2                            # table row (node pair, dup'd)

    key = (core * NWIN + w_in_core) * 2 + j
    cnt = np.bincount(key, minlength=P * NWIN * 2).reshape(P, NWIN, 2)
    cmax = cnt.max(axis=0)                    # (NWIN, 2)
    nch = np.ceil(cmax / 128).astype(np.int64)
    nch = np.maximum(nch, (cmax > 0))

    # chunk layout per window: parity-0 chunks then parity-1 chunks
    chunk_base = np.zeros((NWIN, 2), dtype=np.int64)
    win_plan = []                             # per window: [(chunk, j), ...]
    ch = 0
    for w in range(NWIN):
        entries = []
        for jj in range(2):
            chunk_base[w, jj] = ch
            for _ in range(int(nch[w, jj])):
                entries.append((ch, jj))
                ch += 1
        win_plan.append(entries)
    CH = ch
    NSEG = (CH + SEGC - 1) // SEGC
    CHP = NSEG * SEGC
    NTOKP = CHP * 128

    # sort edges by (core, window, parity, row) -> slots
    skey = key * (NROWS + 1) + row
    order = np.argsort(skey, kind="stable")
    s_key = key[order]
    s_row = row[order]
    s_dloc = dloc[order]
    grp_start = np.zeros(P * NWIN * 2, dtype=np.int64)
    grp_start[1:] = np.cumsum(np.bincount(s_key, minlength=P * NWIN * 2))[:-1]
    pos = np.arange(E) - grp_start[s_key]

    s_core = s_key // (NWIN * 2)
    s_w = (s_key // 2) % NWIN
    s_j = s_key % 2
    slot = s_core * NTOKP + chunk_base[s_w, s_j] * 128 + pos

    tok_row = ((np.arange(P * NTOKP) * 9973) % NROWS).astype(np.int16)
    tok_row[slot] = s_row.astype(np.int16)

    ti = tok_row.reshape(P, NTOKP // 16, 16)
    ti = np.swapaxes(ti, 1, 2)                               # (P, 16, cols)
    gidx_dev = np.tile(ti, (1, 8, 1)).copy()                 # (P, 128, cols)

    # per-token dst-local ids (255 = dummy -> zero one-hot row on device)
    tok_dloc = np.full(P * NTOKP, 255.0, dtype=np.float32)
    tok_dloc[slot] = s_dloc.astype(np.float32)
    td = tok_dloc.reshape(P, CHP, 128)
    dloc_dev = np.ascontiguousarray(
        np.swapaxes(td, 1, 2)).astype(ml_dtypes.bfloat16)    # (P, 128, CHP)

    bat = np.full(N_NODES_REAL, 0, dtype=np.int64)
    bat[:] = bat0
    bat_pad = np.full(NTOT, -1, dtype=np.int64)
    bat_pad[perm] = bat
    B = (bat_pad[:, None] == np.arange(NG)[None, :])
    B_dev = np.ascontiguousarray(
        B.reshape(P, NWIN, 128, NG).transpose(0, 2, 1, 3)
        .reshape(P, 128, NWIN * NG)).astype(ml_dtypes.bfloat16)
    counts = np.bincount(bat0, minlength=NG)[:NG].astype(np.float32)
    invc = (1.0 / np.maximum(counts, 1.0)).reshape(NG, 1)

    return dict(CH=CH, CHP=CHP, NSEG=NSEG, win_plan=win_plan, perm=perm,
                gidx_dev=gidx_dev, dloc_dev=dloc_dev, B_dev=B_dev, invc=invc)


def _pack_x(x):
    xp = np.zeros((NTOT, INF), dtype=np.float32)
    xp[:x.shape[0]] = x
    xc = xp.reshape(P, 2, NPC // 2, INF)
    return np.ascontiguousarray(xc.transpose(0, 1, 3, 2).reshape(P, 128, NPC // 2))


def _remix(W):
    """W: (2, out, in, 4) -> 8 slot matrices (out, in) + bias (out,).

    Basis slots: [sin h, cos h, sin^2 h, (1+sin2h)/2, sin^2 2h,
                  (1-sin4h)/2, sin3h, cos3h]."""
    W0, W1 = W[0], W[1]          # cos / sin coefficient stacks
    slots = [
        W1[:, :, 0],
        W0[:, :, 0],
        -2.0 * W0[:, :, 1],
        2.0 * W1[:, :, 1],
        -2.0 * W0[:, :, 3],
        -2.0 * W1[:, :, 3],
        W1[:, :, 2],
        W0[:, :, 2],
    ]
    bias = (W0[:, :, 1] - W1[:, :, 1] + W0[:, :, 3] + W1[:, :, 3]).sum(axis=1)
    return slots, bias.astype(np.float32)


def _pack_weights(W_in, W_conv, W_out):
    sl_in, b_in = _remix(W_in)
    # input: fused stationary per (half, slot): 128x128 with 64x32 blocks at
    # (rows 0:64 -> out 32*half) and (rows 64:128 -> out 32*(half+2))
    win = np.zeros((128, 2 * 8 * 128), dtype=np.float16)
    for h in range(2):
        for m in range(8):
            c0 = (h * 8 + m) * 128
            win[0:64, c0 + 32 * h:c0 + 32 * h + 32] = (
                sl_in[m].T.astype(np.float16))
            win[64:128, c0 + 32 * (h + 2):c0 + 32 * (h + 2) + 32] = (
                sl_in[m].T.astype(np.float16))
    # conv: fused stationary per (layer, slot): block-diag of 4 identical
    # 32x32 tiles
    wc = np.zeros((128, NCONV * 8 * 128), dtype=np.float16)
    biases = np.zeros((128, 1 + NCONV), dtype=np.float32)
    biases[:, 0] = np.tile(b_in, 4)
    for l in range(NCONV):
        sl, bl = _remix(W_conv[l])
        biases[:, 1 + l] = np.tile(bl, 4)
        for b in range(4):
            for m in range(8):
                c0 = (l * 8 + m) * 128
                wc[32 * b:32 * b + 32, c0 + 32 * b:c0 + 32 * b + 32] = (
                    sl[m].T.astype(np.float16))
    w0r = np.tile(W_out[0, 0, :, 0].astype(np.float32), (128, 1))
    w1r = np.tile(W_out[1, 0, :, 0].astype(np.float32), (128, 1))
    return win, wc, biases, w0r, w1r


# ----------------------------------------------------------------------------
# device program
# ----------------------------------------------------------------------------

def _build(meta):
    CH, CHP, NSEG = meta["CH"], meta["CHP"], meta["NSEG"]
    win_plan = meta["win_plan"]
    XCOLS = NPC // 2                  # 3328
    NTOKP = CHP * 128

    nc = bacc.Bacc("TRN2", target_bir_lowering=False, debug=False,
                   num_devices=P, num_swdge_queues=4)

    x_d = nc.dram_tensor("x_pack", [128, XCOLS], F32, kind="ExternalInput")
    win_d = nc.dram_tensor("win_w", [128, 2 * 8 * 128], F16, kind="ExternalInput")
    wc_d = nc.dram_tensor("wc_w", [128, NCONV * 8 * 128], F16, kind="ExternalInput")
    bias_d = nc.dram_tensor("biases", [128, 1 + NCONV], F32, kind="ExternalInput")
    w0_d = nc.dram_tensor("w0r", [128, HID], F32, kind="ExternalInput")
    w1_d = nc.dram_tensor("w1r", [128, HID], F32, kind="ExternalInput")
    bout_d = nc.dram_tensor("bout", [128, 1], F32, kind="ExternalInput")
    invc_d = nc.dram_tensor("invc", [128, 1], F32, kind="ExternalInput")
    gidx_d = nc.dram_tensor("gidx", [128, NTOKP // 16], I16, kind="ExternalInput")
    dloc_d = nc.dram_tensor("dloc", [128, CHP], BF16, kind="ExternalInput")
    iota_d = nc.dram_tensor("iota", [128, 128], BF16, kind="ExternalInput")
    id32_d = nc.dram_tensor("id32", [128, 32], F32, kind="ExternalInput")
    id32b_d = nc.dram_tensor("id32b", [128, 32], BF16, kind="ExternalInput")
    B_d = nc.dram_tensor("Bmat", [128, NWIN * NG], BF16, kind="ExternalInput")

    out_d = nc.dram_tensor("out", [NG, 1], F32, kind="ExternalOutput")

    AG_GROUPS = [list(range(P))]
    NT = 416
    GBUF = 7

    with tile.TileContext(nc) as tc:
        with (
            tc.tile_pool(name="const", bufs=1) as cp,
            tc.tile_pool(name="feat", bufs=1) as fp,
            tc.tile_pool(name="ftmp", bufs=1) as tp,
            tc.tile_pool(name="work", bufs=1) as wp,
            tc.tile_pool(name="gbuf", bufs=1) as gp,
            tc.tile_pool(name="ohp", bufs=1) as ohp,
            tc.tile_pool(name="pmsg", bufs=2, space="PSUM") as pmsg_p,
            tc.tile_pool(name="ptr", bufs=2, space="PSUM") as ptr_p,
            tc.tile_pool(name="pm", bufs=3, space="PSUM") as pm_p,
            tc.tile_pool(name="ppool", bufs=1, space="PSUM") as ppool_p,
            tc.tile_pool(name="dram", bufs=1, space="DRAM") as dp,
        ):
            # ---- constants ----
            x_sb = wp.tile([128, XCOLS], F32, name="x_sb", tag="bigx")
            nc.sync.dma_start(x_sb[:], x_d[:])
            win_sb = cp.tile([128, 2 * 8 * 128], F16)
            nc.sync.dma_start(win_sb[:], win_d[:])
            wc_sb = cp.tile([128, NCONV * 8 * 128], F16)
            nc.sync.dma_start(wc_sb[:], wc_d[:])
            bias_sb = cp.tile([128, 1 + NCONV], F32)
            nc.sync.dma_start(bias_sb[:], bias_d[:])
            w0_sb = cp.tile([128, HID], F32)
            nc.sync.dma_start(w0_sb[:], w0_d[:])
            w1_sb = cp.tile([128, HID], F32)
            nc.sync.dma_start(w1_sb[:], w1_d[:])
            bout_sb = cp.tile([128, 1], F32)
            nc.sync.dma_start(bout_sb[:], bout_d[:])
            invc_sb = cp.tile([128, 1], F32)
            nc.sync.dma_start(invc_sb[:], invc_d[:])
            gidx_sb = cp.tile([128, NTOKP // 16], I16)
            nc.sync.dma_start(gidx_sb[:], gidx_d[:])
            dloc_sb = cp.tile([128, CHP], BF16)
            nc.sync.dma_start(dloc_sb[:], dloc_d[:])
            iota_sb = cp.tile([128, 128], BF16)
            nc.sync.dma_start(iota_sb[:], iota_d[:])
            id32_sb = cp.tile([128, 32], F32)
            nc.sync.dma_start(id32_sb[:], id32_d[:])
            id32b_sb = cp.tile([128, 32], BF16)
            nc.sync.dma_start(id32b_sb[:], id32b_d[:])
            zb = cp.tile([128, 1], F32)
            nc.vector.memset(zb[:], 0.0)
            m1 = cp.tile([128, 1], F32)
            nc.vector.memset(m1[:], -1.0)

            h_sb = cp.tile([128, BLK], F32)    # packed h^T: partition 32*blk+f

            shard = [dp.tile([NPC // 2, 128], BF16, name=f"shard{l}")
                     for l in range(NCONV)]
            table = [dp.tile([NROWS, 128], BF16, name=f"table{l}",
                             addr_space="Shared")
                     for l in range(NCONV)]
            pool_in = dp.tile([HID, NG], F32)
            pool_out = dp.tile([HID, NG], F32, addr_space="Shared")

            def feat_chain(src, FREE, pfx, row0, nrows):
                """8 f16 basis-feature tiles (rows row0:row0+nrows valid)."""
                ps = slice(row0, row0 + nrows)

                def ts(dst, a, s1, s2, o0, o1=None):
                    if o1 is None:
                        nc.vector.tensor_scalar(dst, a, s1, None, o0)
                    else:
                        nc.vector.tensor_scalar(dst, a, s1, s2, o0, o1)

                def scr(nm, dt=F32):
                    t = tp.tile([128, FREE], dt, name=f"{pfx}{nm}", tag="scr",
                                bufs=3, padded_shape=[128, BLK])
                    return t[ps, :]

                def keep(nm):
                    t = tp.tile([128, FREE], F32, name=f"{pfx}{nm}", tag=nm,
                                bufs=1, padded_shape=[128, BLK])
                    return t[ps, :]

                slots = [fp.tile([128, FREE], F16, name=f"{pfx}slot{i}",
                                 tag=f"feat{i}", padded_shape=[128, BLK])
                         for i in range(8)]
                sl = [s[ps, :] for s in slots]
                zbs, m1s = zb[ps, :], m1[ps, :]

                n0 = scr("n0", I32)
                ts(n0, src, INV_2PI, None, OP.mult)
                nf0 = scr("nf0")
                ts(nf0, n0, -TWO_PI, None, OP.mult)
                r0 = scr("r0")
                nc.vector.tensor_tensor(r0, src, nf0, OP.add)
                n9 = scr("n9", I32)
                ts(n9, src, INV_2PI, 0.25, OP.mult, OP.add)
                nf9 = scr("nf9")
                ts(nf9, n9, -TWO_PI, PI / 2, OP.mult, OP.add)
                r9 = scr("r9")
                nc.vector.tensor_tensor(r9, src, nf9, OP.add)

                s1f = keep("s1f")
                nc.scalar.activation(s1f, r0, AF.Sin, bias=zbs)
                c1f = keep("c1f")
                nc.scalar.activation(c1f, r9, AF.Sin, bias=zbs)
                nc.scalar.activation(sl[0], r0, AF.Sin, bias=zbs)
                nc.scalar.activation(sl[1], r9, AF.Sin, bias=zbs)
                sqsf = keep("sqsf")
                nc.scalar.activation(sqsf, s1f, AF.Square)
                nc.scalar.activation(sl[2], s1f, AF.Square)
                d = scr("d")
                nc.vector.tensor_tensor(d, s1f, c1f, OP.add)
                sqdf = keep("sqdf")
                nc.scalar.activation(sqdf, d, AF.Square, scale=ISQ2)
                nc.scalar.activation(sl[3], d, AF.Square, scale=ISQ2)
                nc.scalar.activation(sl[4], sqdf, AF.Square,
                                     bias=m1s, scale=2.0)
                tc2 = scr("tc2")
                ts(tc2, sqsf, -2.0, 1.0, OP.mult, OP.add)
                ts2 = scr("ts2")
                ts(ts2, sqdf, 2.0, -1.0, OP.mult, OP.add)
                td2 = scr("td2")
                nc.vector.tensor_tensor(td2, tc2, ts2, OP.subtract)
                nc.scalar.activation(sl[5], td2, AF.Square, scale=ISQ2)
                t3 = scr("t3")
                ts(t3, sqsf, -4.0, 3.0, OP.mult, OP.add)
                nc.vector.tensor_tensor(sl[6], s1f, t3, OP.mult)
                t4 = scr("t4")
                ts(t4, sqsf, -4.0, 1.0, OP.mult, OP.add)
                nc.vector.tensor_tensor(sl[7], c1f, t4, OP.mult)
                return slots

            def msg_full(l):
                """Emit msg compute for conv layer l (full width)."""
                mTp = wp.tile([128, BLK], BF16, name=f"mT{l}",
                              tag="mT", bufs=2)
                slots = feat_chain(h_sb[:], BLK, f"m{l}_", 0, 128)
                msg_matmuls(l, slots, mTp, 0)
                msg_stage(l, mTp)

            def msg_matmuls(l, slots, mTp, c0):
                for t4i in range(4):
                    pm2 = pmsg_p.tile([128, NT], F32, name="pm2", tag="pmsg")
                    for m in range(8):
                        nc.tensor.matmul(
                            pm2[:],
                            wc_sb[:, (l * 8 + m) * 128:(l * 8 + m + 1) * 128],
                            slots[m][:, NT * t4i:NT * (t4i + 1)],
                            start=(m == 0), stop=(m == 7),
                        )
                    nc.vector.tensor_scalar(
                        mTp[:, c0 + NT * t4i:c0 + NT * (t4i + 1)], pm2[:],
                        bias_sb[:, 1 + l:2 + l], None, OP.add)

            def msg_stage(l, mTp):
                # reorder columns: even nodes first, then odd, per w2 tile
                mTe = wp.tile([128, BLK], BF16, name=f"mTe{l}",
                              tag="mTe", bufs=2)
                nc.vector.tensor_copy(
                    mTe[:].rearrange("p (w2 sub rh) -> p w2 sub rh",
                                     w2=13, sub=2),
                    mTp[:].rearrange("p (w2 rh sub) -> p w2 sub rh",
                                     w2=13, sub=2))
                for b in range(NBLK):
                    ps = slice(32 * b, 32 * b + 32)
                    stage = wp.tile([128, 13 * HID], BF16, name=f"stg{l}_{b}",
                                    tag="stage", bufs=2)
                    for w2 in range(13):
                        ptr = ptr_p.tile([128, 32], BF16, name="ptrt", tag="ptr")
                        nc.tensor.transpose(
                            ptr[:], mTe[ps, 128 * w2:128 * (w2 + 1)],
                            id32b_sb[ps, :],
                            tile_position=(32 * b, 0))
                        nc.scalar.activation(stage[:, 32 * w2:32 * (w2 + 1)],
                                             ptr[:], AF.Copy)
                    # partitions 0:64 = even rows, 64:128 = odd rows
                    for sub in range(2):
                        nc.sync.dma_start(
                            shard[l][832 * b:832 * (b + 1),
                                     32 * sub:32 * sub + 32].rearrange(
                                "(w2 rh) f -> rh w2 f", w2=13),
                            stage[64 * sub:64 * sub + 64, :].rearrange(
                                "p (w2 f) -> p w2 f", f=HID),
                        )

            # ================= input KAN: x -> h =================
            _s_in = nc.named_scope("ph_input"); _s_in.__enter__()
            for half in range(2):
                xsl = x_sb[:, half * BLK:(half + 1) * BLK]
                slots = feat_chain(xsl, BLK, f"x{half}_", 0, 128)
                for t4i in range(BLK // NT):
                    off = NT * t4i
                    ph = pmsg_p.tile([128, NT], F32, name="ph", tag="pmsg")
                    for m in range(8):
                        nc.tensor.matmul(
                            ph[:],
                            win_sb[:, (half * 8 + m) * 128:(half * 8 + m + 1) * 128],
                            slots[m][:, NT * t4i:NT * (t4i + 1)],
                            start=(m == 0), stop=(m == 7),
                        )
                    for hb in (half, half + 2):
                        nc.vector.tensor_scalar(
                            h_sb[32 * hb:32 * hb + 32, off:off + NT],
                            ph[32 * hb:32 * hb + 32, :],
                            bias_sb[32 * hb:32 * hb + 32, 0:1], None, OP.add)
            msg_full(0)
            _s_in.__exit__(None, None, None)

            # pool resources (consumed per-block during the last scatter)
            B_sb = wp.tile([128, NWIN * NG], BF16, name="B_sb", tag="bigx")
            nc.sync.dma_start(B_sb[:], B_d[:])
            ppool = ppool_p.tile([HID, NG], F32)
            hbf = wp.tile([128, BLK], BF16, name="hbf", tag="hbf", bufs=1)

            def pool_block(b):
                ps = slice(32 * b, 32 * b + 32)
                nc.vector.tensor_copy(hbf[ps, :], h_sb[ps, :])
                for w2 in range(13):
                    w = 13 * b + w2
                    ptb = ptr_p.tile([128, 32], BF16, name="ptb", tag="ptr")
                    nc.tensor.transpose(
                        ptb[:], hbf[ps, 128 * w2:128 * (w2 + 1)],
                        id32b_sb[ps, :], tile_position=(32 * b, 0))
                    htile = wp.tile([128, 32], BF16, name="htile",
                                    tag="htile", bufs=3)
                    nc.scalar.activation(htile[:], ptb[:], AF.Copy)
                    nc.tensor.matmul(
                        ppool[:], htile[:], B_sb[:, NG * w:NG * (w + 1)],
                        start=(w == 0), stop=(w == NWIN - 1),
                    )

            _s_ag = nc.named_scope("ph_ag0"); _s_ag.__enter__()
            nc.gpsimd.collective_compute(
                "AllGather", OP.bypass,
                ins=[shard[0][:]], outs=[table[0][:]],
                replica_groups=AG_GROUPS,
            )
            _s_ag.__exit__(None, None, None)

            # ================= conv layers: scatter (+ pipelined msg) ======
            for l in range(NCONV):
                _s_sc = nc.named_scope(f"ph_scat{l}"); _s_sc.__enter__()
                Gs = [None] * NSEG
                OHs = [None] * NSEG
                state = {"issued": 0, "built": 0}

                def issue_seg(s, l=l, Gs=Gs):
                    G = gp.tile([128, SEGC, 128], BF16, name=f"G{l}_{s}",
                                tag=f"G{s % GBUF}")
                    nc.gpsimd.dma_gather(
                        G[:], table[l][:],
                        gidx_sb[:, s * (SEGC * 8):(s + 1) * (SEGC * 8)],
                        num_idxs=SEGC * 128, num_idxs_reg=SEGC * 128,
                        elem_size=128, single_packet=False, queue_num=s % 4,
                    )
                    Gs[s] = G

                iota_b = iota_sb[:].rearrange("p (x d) -> p x d", x=1)

                def load_oh(s, l=l, OHs=OHs):
                    oh = ohp.tile([128, SEGC, 128], BF16, name=f"oh{l}_{s}",
                                  tag=f"oh{s % 4}")
                    for hseg in range(2):
                        c0 = s * SEGC + hseg * (SEGC // 2)
                        nc.vector.tensor_tensor(
                            oh[:, hseg * (SEGC // 2):(hseg + 1) * (SEGC // 2), :],
                            iota_b.to_broadcast([128, SEGC // 2, 128]),
                            dloc_sb[:, c0:c0 + SEGC // 2]
                            .to_broadcast([128, SEGC // 2, 128]),
                            OP.is_equal)
                    OHs[s] = oh

                for w in range(NWIN):
                    entries = win_plan[w]
                    if entries:
                        cg = entries[0][0]
                        g_need = min(cg // SEGC + (GBUF - 1), NSEG - 1)
                        while state["issued"] <= g_need:
                            issue_seg(state["issued"])
                            state["issued"] += 1
                        oh_need = min((cg + len(entries) - 1) // SEGC + 1,
                                      NSEG - 1)
                        while state["built"] <= oh_need:
                            load_oh(state["built"])
                            state["built"] += 1
                        hb, off = (w * WIN) // BLK, (w * WIN) % BLK
                        pm = pm_p.tile([128, WIN], F32, name="pmw", tag="pm")
                        pms = pm[32 * hb:32 * hb + 32, :]
                        nent = len(entries)
                        for i, (c, jj) in enumerate(entries):
                            s, cl = c // SEGC, c % SEGC
                            nc.tensor.matmul(
                                pms, Gs[s][:, cl, 32 * jj:32 * jj + 32],
                                OHs[s][:, cl, :],
                                start=(i == 0), stop=(i == nent - 1),
                                tile_position=(0, 32 * hb),
                            )
                        hsl = h_sb[32 * hb:32 * hb + 32, off:off + WIN]
                        nc.vector.tensor_tensor(hsl, pms, hsl, OP.add)
                # leaky relu: h = max(z, 0.01*z)
                lrt = wp.tile([128, BLK], F32, name=f"lr{l}",
                              tag="lrt", bufs=2)
                nc.vector.tensor_scalar(lrt[:], h_sb[:], NEG, None, OP.mult)
                nc.vector.tensor_tensor(h_sb[:], h_sb[:], lrt[:], OP.max)
                if l + 1 < NCONV:
                    msg_full(l + 1)
                else:
                    for b in range(NBLK):
                        pool_block(b)
                _s_sc.__exit__(None, None, None)
                if l + 1 < NCONV:
                    _s_ag1 = nc.named_scope(f"ph_ag{l+1}"); _s_ag1.__enter__()
                    nc.gpsimd.collective_compute(
                        "AllGather", OP.bypass,
                        ins=[shard[l + 1][:]], outs=[table[l + 1][:]],
                        replica_groups=AG_GROUPS,
                    )
                    _s_ag1.__exit__(None, None, None)

            # ================= pool + readout =================
            _s_po = nc.named_scope("ph_pool"); _s_po.__enter__()
            pool_sb = wp.tile([HID, NG], F32, name="pool_sb")
            nc.vector.tensor_copy(pool_sb[:], ppool[:])
            nc.sync.dma_start(pool_in[:], pool_sb[:])
            nc.gpsimd.collective_compute(
                "AllReduce", OP.add,
                ins=[pool_in[:]], outs=[pool_out[:]],
                replica_groups=AG_GROUPS,
            )
            psum_sb = wp.tile([HID, NG], F32, name="psum_sb")
            nc.sync.dma_start(psum_sb[:], pool_out[:])
            ptry = ptr_p.tile([128, 32], F32, name="ptry", tag="ptr")
            nc.tensor.transpose(ptry[:], psum_sb[:], id32_sb[0:32, :])
            y_sb = wp.tile([NG, HID], F32, name="y_sb")
            nc.vector.tensor_scalar(y_sb[:], ptry[:], invc_sb[:], None, OP.mult)

            # readout: sin(y), cos(y) via the same range reduction
            def sincos(src, pfx, quarter):
                n = wp.tile([NG, HID], I32, name=f"{pfx}n")
                nf = wp.tile([NG, HID], F32, name=f"{pfx}nf")
                if quarter:
                    nc.vector.tensor_scalar(n[:], src, INV_2PI, 0.25, OP.mult, OP.add)
                    nc.vector.tensor_scalar(nf[:], n[:], -TWO_PI, PI / 2,
                                            OP.mult, OP.add)
                else:
                    nc.vector.tensor_scalar(n[:], src, INV_2PI, None, OP.mult)
                    nc.vector.tensor_scalar(nf[:], n[:], -TWO_PI, None, OP.mult)
                r = wp.tile([NG, HID], F32, name=f"{pfx}r")
                nc.vector.tensor_tensor(r[:], src, nf[:], OP.add)
                o = wp.tile([NG, HID], F32, name=f"{pfx}o")
                nc.scalar.activation(o[:], r[:], AF.Sin, bias=zb[:])
                return o

            sin_y = sincos(y_sb[:], "sy", False)
            cos_y = sincos(y_sb[:], "cy", True)
            nc.vector.tensor_tensor(cos_y[:], cos_y[:], w0_sb[:], OP.mult)
            nc.vector.tensor_tensor(sin_y[:], sin_y[:], w1_sb[:], OP.mult)
            nc.vector.tensor_tensor(cos_y[:], cos_y[:], sin_y[:], OP.add)
            red = wp.tile([NG, 1], F32, name="red")
            nc.vector.tensor_reduce(red[:], cos_y[:], mybir.AxisListType.X, OP.add)
            o_sb = wp.tile([NG, 1], F32, name="o_sb")
            nc.scalar.activation(o_sb[:], red[:], AF.Sigmoid, bias=bout_sb[:])
            nc.sync.dma_start(out_d[:], o_sb[:])
            _s_po.__exit__(None, None, None)

    nc.compile()
    return nc


# ----------------------------------------------------------------------------
# entry point
# ----------------------------------------------------------------------------

def kernel(x, edge_index, batch, W_in, W_conv, W_out, b_out):
    global LAST_RESULTS
    x = np.asarray(x, dtype=np.float32)
    W_in = np.asarray(W_in, dtype=np.float32)
    W_conv = np.asarray(W_conv, dtype=np.float32)
    W_out = np.asarray(W_out, dtype=np.float32)
    b_out = np.asarray(b_out, dtype=np.float32)

    meta = _prep(edge_index, batch)
    perm = meta["perm"]
    x_perm = np.zeros((NTOT, INF), dtype=np.float32)
    x_perm[perm] = x
    x_pack = _pack_x(x_perm)
    win, wc, biases, w0r, w1r = _pack_weights(W_in, W_conv, W_out)

    nc = _build(meta)

    iota = np.tile(np.arange(128, dtype=np.float32)[None, :],
                   (128, 1)).astype(ml_dtypes.bfloat16)
    id32 = np.tile(np.eye(32, dtype=np.float32), (4, 1))
    id32b = np.tile(np.eye(32, dtype=ml_dtypes.bfloat16), (4, 1))
    bout_col = np.full((128, 1), float(b_out.ravel()[0]), dtype=np.float32)

    in_maps = []
    for c in range(P):
        in_maps.append({
            "x_pack": x_pack[c],
            "win_w": win,
            "wc_w": wc,
            "biases": biases,
            "w0r": w0r,
            "w1r": w1r,
            "bout": bout_col,
            "invc": meta["invc"].astype(np.float32),
            "gidx": meta["gidx_dev"][c],
            "dloc": meta["dloc_dev"][c],
            "iota": iota,
            "id32": id32,
            "id32b": id32b,
            "Bmat": meta["B_dev"][c],
        })

    import os as _os
    _tc = _os.environ.get("TRACE_CORES")
    _kw = {}
    if _tc:
        _kw = dict(trace_cores=[int(x) for x in _tc.split(",")], stitch_traces=True)
    res = run_bass_kernel_spmd(nc, in_maps, core_ids=list(range(P)), **_kw)
    LAST_RESULTS = res
    return np.asarray(res.results[0]["out"], dtype=np.float32)


# revision 27
# speedup vs baseline: 1.0238x; 1.0082x over previous
"""KA-GNN (Fourier-KAN message passing) on 8 Trainium2 NeuronCores — v3.

Sharding: nodes/edges partitioned by destination across 8 cores, with a
host-side node permutation that bin-packs nodes into 128-dst windows by
in-degree so every (core, window) has ~equal edge count. Per conv layer
each core computes its msg shard (node-wise Fourier-KAN) in bf16 packed
2-nodes-per-256B-row (node pair duplicated to satisfy the 256B gather
granularity), an AllGather builds the full msg table in DRAM, then async
dma_gather segments (rotating the 4 SWDGE queues so their drains overlap)
pull per-edge source rows. Host-precomputed one-hot matrices (DMA'd from
DRAM, no on-device IS_EQ) drive bf16 scatter-matmuls accumulating each
128-dst window in PSUM. The next layer's msg compute is emitted per
32-feature block as soon as that block's windows finish, hiding it under
the scatter. Pool via one-hot matmul + small AllReduce; readout + sigmoid
on device.

The Fourier features sin/cos(k*h), k=1..4 are built from sin(h), cos(h)
(range-reduced via round-to-nearest f32->i32 cast) plus ScalarE Square
chains; the k-harmonics are linear in 8 basis tensors, so the KAN weights
are remixed host-side onto that basis (plus a per-output bias column).
"""

import heapq
import math
import numpy as np
import ml_dtypes

import concourse.bacc as bacc
import concourse.mybir as mybir
import concourse.tile as tile
from concourse.bass_utils import run_bass_kernel_spmd

F32 = mybir.dt.float32
BF16 = mybir.dt.bfloat16
I16 = mybir.dt.int16
I32 = mybir.dt.int32
F16 = mybir.dt.float16
AF = mybir.ActivationFunctionType
OP = mybir.AluOpType

P = 8
HID = 32
INF = 64
NG = 128
NCONV = 2
NEG = 0.01

NPC = 6656                 # nodes per core (padded total 53248)
NTOT = NPC * P
NBLK = 4
BLK = NPC // NBLK          # 1664
WIN = 128
NWIN = NPC // WIN          # 52
N_NODES_REAL = 50000
NROWS = NTOT // 2          # 2 nodes per 256B bf16 row (pair duplicated)
SEGC = 16                  # chunks per gather segment (2048 tokens)

TWO_PI = float(2 * math.pi)
PI = float(math.pi)
INV_2PI = float(1.0 / (2 * math.pi))
ISQ2 = float(1.0 / math.sqrt(2.0))

LAST_RESULTS = None        # test.py reads exec_time_ns from here


def _install_ntff_hook():
    # restore the axon NTFF profiling hook when the image's antenv lacks it
    import sys
    import types
    try:
        import antenv.axon_hooks  # noqa: F401
        return
    except ImportError:
        pass
    try:
        import antenv
        from trn_agent_boot.trn_boot import _ntff_profile_via_ctypes
        hook = _ntff_profile_via_ctypes("/opt/axon/libaxon_pjrt.so")
        mod = types.ModuleType("antenv.axon_hooks")
        holder = {"h": hook}
        mod.set_axon_ntff_profile_hook = lambda h: holder.__setitem__("h", h)
        mod.get_axon_ntff_profile_hook = lambda: holder["h"]
        sys.modules["antenv.axon_hooks"] = mod
        antenv.axon_hooks = mod
    except Exception:
        pass


_install_ntff_hook()


# ----------------------------------------------------------------------------
# host-side sharding / index prep
# ----------------------------------------------------------------------------

def _balance_nodes(dst):
    """Bin-pack real nodes into P*NWIN windows of <=128 so every window's
    in-edge count is ~equal. Returns perm: old node id -> new node id."""
    indeg = np.bincount(dst, minlength=N_NODES_REAL).astype(np.int64)
    order = np.argsort(-indeg, kind="stable")
    nw = P * NWIN
    heap = [(0, 0, w) for w in range(nw)]   # (sum, count, window)
    heapq.heapify(heap)
    wslot = np.empty(N_NODES_REAL, dtype=np.int64)
    wcnt = np.zeros(nw, dtype=np.int64)
    for i in order:
        s, c, w = heapq.heappop(heap)
        wslot[i] = w
        cpos = wcnt[w]
        wcnt[w] += 1
        if c + 1 < WIN:
            heapq.heappush(heap, (s + indeg[i], c + 1, w))
        # record position later via stable counting
    # positions: stable order of assignment per window
    perm = np.empty(N_NODES_REAL, dtype=np.int64)
    pos_in_w = np.zeros(nw, dtype=np.int64)
    for i in order:
        w = wslot[i]
        core, wl = w // NWIN, w % NWIN
        perm[i] = core * NPC + wl * WIN + pos_in_w[w]
        pos_in_w[w] += 1
    return perm


def _prep(edge_index, batch):
    src0 = np.asarray(edge_index[0], dtype=np.int64)
    dst0 = np.asarray(edge_index[1], dtype=np.int64)
    bat0 = np.asarray(batch, dtype=np.int64)
    E = src0.shape[0]

    perm = _balance_nodes(dst0)
    src = perm[src0]
    dst = perm[dst0]

    core = dst // NPC
    w_in_core = (dst % NPC) // WIN
    dloc = dst % WIN
    j = src % 2                               # parity: 32-col slice of row
    row = src // 2                            # table row (node pair, dup'd)

    key = (core * NWIN + w_in_core) * 2 + j
    cnt = np.bincount(key, minlength=P * NWIN * 2).reshape(P, NWIN, 2)
    cmax = cnt.max(axis=0)                    # (NWIN, 2)
    nch = np.ceil(cmax / 128).astype(np.int64)
    nch = np.maximum(nch, (cmax > 0))

    # chunk layout per window: parity-0 chunks then parity-1 chunks
    chunk_base = np.zeros((NWIN, 2), dtype=np.int64)
    win_plan = []                             # per window: [(chunk, j), ...]
    ch = 0
    for w in range(NWIN):
        entries = []
        for jj in range(2):
            chunk_base[w, jj] = ch
            for _ in range(int(nch[w, jj])):
                entries.append((ch, jj))
                ch += 1
        win_plan.append(entries)
    CH = ch
    NSEG = (CH + SEGC - 1) // SEGC
    CHP = NSEG * SEGC
    NTOKP = CHP * 128

    # sort edges by (core, window, parity, row) -> slots
    skey = key * (NROWS + 1) + row
    order = np.argsort(skey, kind="stable")
    s_key = key[order]
    s_row = row[order]
    s_dloc = dloc[order]
    grp_start = np.zeros(P * NWIN * 2, dtype=np.int64)
    grp_start[1:] = np.cumsum(np.bincount(s_key, minlength=P * NWIN * 2))[:-1]
    pos = np.arange(E) - grp_start[s_key]

    s_core = s_key // (NWIN * 2)
    s_w = (s_key // 2) % NWIN
    s_j = s_key % 2
    slot = s_core * NTOKP + chunk_base[s_w, s_j] * 128 + pos

    tok_row = ((np.arange(P * NTOKP) * 9973) % NROWS).astype(np.int16)
    tok_row[slot] = s_row.astype(np.int16)

    ti = tok_row.reshape(P, NTOKP // 16, 16)
    ti = np.swapaxes(ti, 1, 2)                               # (P, 16, cols)
    gidx_dev = np.tile(ti, (1, 8, 1)).copy()                 # (P, 128, cols)

    # per-token dst-local ids (255 = dummy -> zero one-hot row on device)
    tok_dloc = np.full(P * NTOKP, 255.0, dtype=np.float32)
    tok_dloc[slot] = s_dloc.astype(np.float32)
    td = tok_dloc.reshape(P, CHP, 128)
    dloc_dev = np.ascontiguousarray(
        np.swapaxes(td, 1, 2)).astype(ml_dtypes.bfloat16)    # (P, 128, CHP)

    bat = np.full(N_NODES_REAL, 0, dtype=np.int64)
    bat[:] = bat0
    bat_pad = np.full(NTOT, -1, dtype=np.int64)
    bat_pad[perm] = bat
    B = (bat_pad[:, None] == np.arange(NG)[None, :])
    B_dev = np.ascontiguousarray(
        B.reshape(P, NWIN, 128, NG).transpose(0, 2, 1, 3)
        .reshape(P, 128, NWIN * NG)).astype(ml_dtypes.bfloat16)
    counts = np.bincount(bat0, minlength=NG)[:NG].astype(np.float32)
    invc = (1.0 / np.maximum(counts, 1.0)).reshape(NG, 1)

    return dict(CH=CH, CHP=CHP, NSEG=NSEG, win_plan=win_plan, perm=perm,
                gidx_dev=gidx_dev, dloc_dev=dloc_dev, B_dev=B_dev, invc=invc)


def _pack_x(x):
    xp = np.zeros((NTOT, INF), dtype=np.float32)
    xp[:x.shape[0]] = x
    xc = xp.reshape(P, 2, NPC // 2, INF)
    return np.ascontiguousarray(xc.transpose(0, 1, 3, 2).reshape(P, 128, NPC // 2))


def _remix(W):
    """W: (2, out, in, 4) -> 8 slot matrices (out, in) + bias (out,).

    Basis slots: [sin h, cos h, sin^2 h, (1+sin2h)/2, sin^2 2h,
                  (1-sin4h)/2, sin3h, cos3h]."""
    W0, W1 = W[0], W[1]          # cos / sin coefficient stacks
    slots = [
        W1[:, :, 0],
        W0[:, :, 0],
        -2.0 * W0[:, :, 1],
        2.0 * W1[:, :, 1],
        -2.0 * W0[:, :, 3],
        -2.0 * W1[:, :, 3],
        W1[:, :, 2],
        W0[:, :, 2],
    ]
    bias = (W0[:, :, 1] - W1[:, :, 1] + W0[:, :, 3] + W1[:, :, 3]).sum(axis=1)
    return slots, bias.astype(np.float32)


def _pack_weights(W_in, W_conv, W_out):
    sl_in, b_in = _remix(W_in)
    # input: fused stationary per (half, slot): 128x128 with 64x32 blocks at
    # (rows 0:64 -> out 32*half) and (rows 64:128 -> out 32*(half+2))
    win = np.zeros((128, 2 * 8 * 128), dtype=np.float16)
    for h in range(2):
        for m in range(8):
            c0 = (h * 8 + m) * 128
            win[0:64, c0 + 32 * h:c0 + 32 * h + 32] = (
                sl_in[m].T.astype(np.float16))
            win[64:128, c0 + 32 * (h + 2):c0 + 32 * (h + 2) + 32] = (
                sl_in[m].T.astype(np.float16))
    # conv: fused stationary per (layer, slot): block-diag of 4 identical
    # 32x32 tiles
    wc = np.zeros((128, NCONV * 8 * 128), dtype=np.float16)
    biases = np.zeros((128, 1 + NCONV), dtype=np.float32)
    biases[:, 0] = np.tile(b_in, 4)
    for l in range(NCONV):
        sl, bl = _remix(W_conv[l])
        biases[:, 1 + l] = np.tile(bl, 4)
        for b in range(4):
            for m in range(8):
                c0 = (l * 8 + m) * 128
                wc[32 * b:32 * b + 32, c0 + 32 * b:c0 + 32 * b + 32] = (
                    sl[m].T.astype(np.float16))
    w0r = np.tile(W_out[0, 0, :, 0].astype(np.float32), (128, 1))
    w1r = np.tile(W_out[1, 0, :, 0].astype(np.float32), (128, 1))
    return win, wc, biases, w0r, w1r


# ----------------------------------------------------------------------------
# device program
# ----------------------------------------------------------------------------

def _build(meta):
    CH, CHP, NSEG = meta["CH"], meta["CHP"], meta["NSEG"]
    win_plan = meta["win_plan"]
    XCOLS = NPC // 2                  # 3328
    NTOKP = CHP * 128

    nc = bacc.Bacc("TRN2", target_bir_lowering=False, debug=False,
                   num_devices=P, num_swdge_queues=4)

    x_d = nc.dram_tensor("x_pack", [128, XCOLS], F32, kind="ExternalInput")
    win_d = nc.dram_tensor("win_w", [128, 2 * 8 * 128], F16, kind="ExternalInput")
    wc_d = nc.dram_tensor("wc_w", [128, NCONV * 8 * 128], F16, kind="ExternalInput")
    bias_d = nc.dram_tensor("biases", [128, 1 + NCONV], F32, kind="ExternalInput")
    w0_d = nc.dram_tensor("w0r", [128, HID], F32, kind="ExternalInput")
    w1_d = nc.dram_tensor("w1r", [128, HID], F32, kind="ExternalInput")
    bout_d = nc.dram_tensor("bout", [128, 1], F32, kind="ExternalInput")
    invc_d = nc.dram_tensor("invc", [128, 1], F32, kind="ExternalInput")
    gidx_d = nc.dram_tensor("gidx", [128, NTOKP // 16], I16, kind="ExternalInput")
    dloc_d = nc.dram_tensor("dloc", [128, CHP], BF16, kind="ExternalInput")
    iota_d = nc.dram_tensor("iota", [128, 128], BF16, kind="ExternalInput")
    id32_d = nc.dram_tensor("id32", [128, 32], F32, kind="ExternalInput")
    id32b_d = nc.dram_tensor("id32b", [128, 32], BF16, kind="ExternalInput")
    B_d = nc.dram_tensor("Bmat", [128, NWIN * NG], BF16, kind="ExternalInput")

    out_d = nc.dram_tensor("out", [NG, 1], F32, kind="ExternalOutput")

    AG_GROUPS = [list(range(P))]
    NT = 416
    GBUF = 7

    with tile.TileContext(nc) as tc:
        with (
            tc.tile_pool(name="const", bufs=1) as cp,
            tc.tile_pool(name="feat", bufs=1) as fp,
            tc.tile_pool(name="ftmp", bufs=1) as tp,
            tc.tile_pool(name="work", bufs=1) as wp,
            tc.tile_pool(name="gbuf", bufs=1) as gp,
            tc.tile_pool(name="ohp", bufs=1) as ohp,
            tc.tile_pool(name="pmsg", bufs=2, space="PSUM") as pmsg_p,
            tc.tile_pool(name="ptr", bufs=2, space="PSUM") as ptr_p,
            tc.tile_pool(name="pm", bufs=3, space="PSUM") as pm_p,
            tc.tile_pool(name="ppool", bufs=1, space="PSUM") as ppool_p,
            tc.tile_pool(name="dram", bufs=1, space="DRAM") as dp,
        ):
            # ---- constants ----
            x_sb = wp.tile([128, XCOLS], F32, name="x_sb", tag="bigx")
            nc.sync.dma_start(x_sb[:], x_d[:])
            win_sb = cp.tile([128, 2 * 8 * 128], F16)
            nc.sync.dma_start(win_sb[:], win_d[:])
            wc_sb = cp.tile([128, NCONV * 8 * 128], F16)
            nc.sync.dma_start(wc_sb[:], wc_d[:])
            bias_sb = cp.tile([128, 1 + NCONV], F32)
            nc.sync.dma_start(bias_sb[:], bias_d[:])
            w0_sb = cp.tile([128, HID], F32)
            nc.sync.dma_start(w0_sb[:], w0_d[:])
            w1_sb = cp.tile([128, HID], F32)
            nc.sync.dma_start(w1_sb[:], w1_d[:])
            bout_sb = cp.tile([128, 1], F32)
            nc.sync.dma_start(bout_sb[:], bout_d[:])
            invc_sb = cp.tile([128, 1], F32)
            nc.sync.dma_start(invc_sb[:], invc_d[:])
            gidx_sb = cp.tile([128, NTOKP // 16], I16)
            nc.sync.dma_start(gidx_sb[:], gidx_d[:])
            dloc_sb = cp.tile([128, CHP], BF16)
            nc.sync.dma_start(dloc_sb[:], dloc_d[:])
            iota_sb = cp.tile([128, 128], BF16)
            nc.sync.dma_start(iota_sb[:], iota_d[:])
            id32_sb = cp.tile([128, 32], F32)
            nc.sync.dma_start(id32_sb[:], id32_d[:])
            id32b_sb = cp.tile([128, 32], BF16)
            nc.sync.dma_start(id32b_sb[:], id32b_d[:])
            zb = cp.tile([128, 1], F32)
            nc.vector.memset(zb[:], 0.0)
            m1 = cp.tile([128, 1], F32)
            nc.vector.memset(m1[:], -1.0)

            h_sb = cp.tile([128, BLK], F32)    # packed h^T: partition 32*blk+f

            shard = [dp.tile([NPC // 2, 128], BF16, name=f"shard{l}")
                     for l in range(NCONV)]
            table = [dp.tile([NROWS, 128], BF16, name=f"table{l}",
                             addr_space="Shared")
                     for l in range(NCONV)]
            pool_in = dp.tile([HID, NG], F32)
            pool_out = dp.tile([HID, NG], F32, addr_space="Shared")

            def feat_chain(src, FREE, pfx, row0, nrows):
                """8 f16 basis-feature tiles (rows row0:row0+nrows valid)."""
                ps = slice(row0, row0 + nrows)

                def ts(dst, a, s1, s2, o0, o1=None):
                    if o1 is None:
                        nc.vector.tensor_scalar(dst, a, s1, None, o0)
                    else:
                        nc.vector.tensor_scalar(dst, a, s1, s2, o0, o1)

                def scr(nm, dt=F32):
                    t = tp.tile([128, FREE], dt, name=f"{pfx}{nm}", tag="scr",
                                bufs=3, padded_shape=[128, BLK])
                    return t[ps, :]

                def keep(nm):
                    t = tp.tile([128, FREE], F32, name=f"{pfx}{nm}", tag=nm,
                                bufs=1, padded_shape=[128, BLK])
                    return t[ps, :]

                slots = [fp.tile([128, FREE], F16, name=f"{pfx}slot{i}",
                                 tag=f"feat{i}", padded_shape=[128, BLK])
                         for i in range(8)]
                sl = [s[ps, :] for s in slots]
                zbs, m1s = zb[ps, :], m1[ps, :]

                n0 = scr("n0", I32)
                ts(n0, src, INV_2PI, None, OP.mult)
                nf0 = scr("nf0")
                ts(nf0, n0, -TWO_PI, None, OP.mult)
                r0 = scr("r0")
                nc.vector.tensor_tensor(r0, src, nf0, OP.add)
                n9 = scr("n9", I32)
                ts(n9, src, INV_2PI, 0.25, OP.mult, OP.add)
                nf9 = scr("nf9")
                ts(nf9, n9, -TWO_PI, PI / 2, OP.mult, OP.add)
                r9 = scr("r9")
                nc.vector.tensor_tensor(r9, src, nf9, OP.add)

                nc.scalar.activation(sl[0], r0, AF.Sin, bias=zbs)
                nc.scalar.activation(sl[1], r9, AF.Sin, bias=zbs)
                nc.scalar.activation(sl[2], sl[0], AF.Square)
                d = scr("d")
                nc.vector.tensor_tensor(d, sl[0], sl[1], OP.add)
                nc.scalar.activation(sl[3], d, AF.Square, scale=ISQ2)
                nc.scalar.activation(sl[4], sl[3], AF.Square,
                                     bias=m1s, scale=2.0)
                tc2 = scr("tc2")
                ts(tc2, sl[2], -2.0, 1.0, OP.mult, OP.add)
                ts2 = scr("ts2")
                ts(ts2, sl[3], 2.0, -1.0, OP.mult, OP.add)
                td2 = scr("td2")
                nc.vector.tensor_tensor(td2, tc2, ts2, OP.subtract)
                nc.scalar.activation(sl[5], td2, AF.Square, scale=ISQ2)
                t3 = scr("t3", F16)
                ts(t3, sl[2], -4.0, 3.0, OP.mult, OP.add)
                nc.vector.tensor_tensor(sl[6], sl[0], t3, OP.mult)
                t4 = scr("t4", F16)
                ts(t4, sl[2], -4.0, 1.0, OP.mult, OP.add)
                nc.vector.tensor_tensor(sl[7], sl[1], t4, OP.mult)
                return slots

            def msg_full(l):
                """Emit msg compute for conv layer l (full width)."""
                mTp = wp.tile([128, BLK], BF16, name=f"mT{l}",
                              tag="mT", bufs=2)
                slots = feat_chain(h_sb[:], BLK, f"m{l}_", 0, 128)
                msg_matmuls(l, slots, mTp, 0)
                msg_stage(l, mTp)

            def msg_matmuls(l, slots, mTp, c0):
                for t4i in range(4):
                    pm2 = pmsg_p.tile([128, NT], F32, name="pm2", tag="pmsg")
                    for m in range(8):
                        nc.tensor.matmul(
                            pm2[:],
                            wc_sb[:, (l * 8 + m) * 128:(l * 8 + m + 1) * 128],
                            slots[m][:, NT * t4i:NT * (t4i + 1)],
                            start=(m == 0), stop=(m == 7),
                        )
                    nc.vector.tensor_scalar(
                        mTp[:, c0 + NT * t4i:c0 + NT * (t4i + 1)], pm2[:],
                        bias_sb[:, 1 + l:2 + l], None, OP.add)

            def msg_stage(l, mTp):
                # reorder columns: even nodes first, then odd, per w2 tile
                mTe = wp.tile([128, BLK], BF16, name=f"mTe{l}",
                              tag="mTe", bufs=2)
                nc.vector.tensor_copy(
                    mTe[:].rearrange("p (w2 sub rh) -> p w2 sub rh",
                                     w2=13, sub=2),
                    mTp[:].rearrange("p (w2 rh sub) -> p w2 sub rh",
                                     w2=13, sub=2))
                for b in range(NBLK):
                    ps = slice(32 * b, 32 * b + 32)
                    stage = wp.tile([128, 13 * HID], BF16, name=f"stg{l}_{b}",
                                    tag="stage", bufs=2)
                    for w2 in range(13):
                        ptr = ptr_p.tile([128, 32], BF16, name="ptrt", tag="ptr")
                        nc.tensor.transpose(
                            ptr[:], mTe[ps, 128 * w2:128 * (w2 + 1)],
                            id32b_sb[ps, :],
                            tile_position=(32 * b, 0))
                        nc.scalar.activation(stage[:, 32 * w2:32 * (w2 + 1)],
                                             ptr[:], AF.Copy)
                    # partitions 0:64 = even rows, 64:128 = odd rows
                    for sub in range(2):
                        nc.sync.dma_start(
                            shard[l][832 * b:832 * (b + 1),
                                     32 * sub:32 * sub + 32].rearrange(
                                "(w2 rh) f -> rh w2 f", w2=13),
                            stage[64 * sub:64 * sub + 64, :].rearrange(
                                "p (w2 f) -> p w2 f", f=HID),
                        )

            # ================= input KAN: x -> h =================
            _s_in = nc.named_scope("ph_input"); _s_in.__enter__()
            for half in range(2):
                xsl = x_sb[:, half * BLK:(half + 1) * BLK]
                slots = feat_chain(xsl, BLK, f"x{half}_", 0, 128)
                for t4i in range(BLK // NT):
                    off = NT * t4i
                    ph = pmsg_p.tile([128, NT], F32, name="ph", tag="pmsg")
                    for m in range(8):
                        nc.tensor.matmul(
                            ph[:],
                            win_sb[:, (half * 8 + m) * 128:(half * 8 + m + 1) * 128],
                            slots[m][:, NT * t4i:NT * (t4i + 1)],
                            start=(m == 0), stop=(m == 7),
                        )
                    for hb in (half, half + 2):
                        nc.vector.tensor_scalar(
                            h_sb[32 * hb:32 * hb + 32, off:off + NT],
                            ph[32 * hb:32 * hb + 32, :],
                            bias_sb[32 * hb:32 * hb + 32, 0:1], None, OP.add)
            msg_full(0)
            _s_in.__exit__(None, None, None)

            # pool resources (consumed per-block during the last scatter)
            B_sb = wp.tile([128, NWIN * NG], BF16, name="B_sb", tag="bigx")
            nc.sync.dma_start(B_sb[:], B_d[:])
            ppool = ppool_p.tile([HID, NG], F32)
            hbf = wp.tile([128, BLK], BF16, name="hbf", tag="hbf", bufs=1)

            def pool_block(b):
                ps = slice(32 * b, 32 * b + 32)
                nc.vector.tensor_copy(hbf[ps, :], h_sb[ps, :])
                for w2 in range(13):
                    w = 13 * b + w2
                    ptb = ptr_p.tile([128, 32], BF16, name="ptb", tag="ptr")
                    nc.tensor.transpose(
                        ptb[:], hbf[ps, 128 * w2:128 * (w2 + 1)],
                        id32b_sb[ps, :], tile_position=(32 * b, 0))
                    htile = wp.tile([128, 32], BF16, name="htile",
                                    tag="htile", bufs=3)
                    nc.scalar.activation(htile[:], ptb[:], AF.Copy)
                    nc.tensor.matmul(
                        ppool[:], htile[:], B_sb[:, NG * w:NG * (w + 1)],
                        start=(w == 0), stop=(w == NWIN - 1),
                    )

            _s_ag = nc.named_scope("ph_ag0"); _s_ag.__enter__()
            nc.gpsimd.collective_compute(
                "AllGather", OP.bypass,
                ins=[shard[0][:]], outs=[table[0][:]],
                replica_groups=AG_GROUPS,
            )
            _s_ag.__exit__(None, None, None)

            # ================= conv layers: scatter (+ pipelined msg) ======
            for l in range(NCONV):
                _s_sc = nc.named_scope(f"ph_scat{l}"); _s_sc.__enter__()
                Gs = [None] * NSEG
                OHs = [None] * NSEG
                state = {"issued": 0, "built": 0}

                def issue_seg(s, l=l, Gs=Gs):
                    G = gp.tile([128, SEGC, 128], BF16, name=f"G{l}_{s}",
                                tag=f"G{s % GBUF}")
                    nc.gpsimd.dma_gather(
                        G[:], table[l][:],
                        gidx_sb[:, s * (SEGC * 8):(s + 1) * (SEGC * 8)],
                        num_idxs=SEGC * 128, num_idxs_reg=SEGC * 128,
                        elem_size=128, single_packet=False, queue_num=s % 4,
                    )
                    Gs[s] = G

                iota_b = iota_sb[:].rearrange("p (x d) -> p x d", x=1)

                def load_oh(s, l=l, OHs=OHs):
                    oh = ohp.tile([128, SEGC, 128], BF16, name=f"oh{l}_{s}",
                                  tag=f"oh{s % 4}")
                    for hseg in range(2):
                        c0 = s * SEGC + hseg * (SEGC // 2)
                        nc.vector.tensor_tensor(
                            oh[:, hseg * (SEGC // 2):(hseg + 1) * (SEGC // 2), :],
                            iota_b.to_broadcast([128, SEGC // 2, 128]),
                            dloc_sb[:, c0:c0 + SEGC // 2]
                            .to_broadcast([128, SEGC // 2, 128]),
                            OP.is_equal)
                    OHs[s] = oh

                for w in range(NWIN):
                    entries = win_plan[w]
                    if entries:
                        cg = entries[0][0]
                        g_need = min(cg // SEGC + (GBUF - 1), NSEG - 1)
                        while state["issued"] <= g_need:
                            issue_seg(state["issued"])
                            state["issued"] += 1
                        oh_need = min((cg + len(entries) - 1) // SEGC + 1,
                                      NSEG - 1)
                        while state["built"] <= oh_need:
                            load_oh(state["built"])
                            state["built"] += 1
                        hb, off = (w * WIN) // BLK, (w * WIN) % BLK
                        pm = pm_p.tile([128, WIN], F32, name="pmw", tag="pm")
                        pms = pm[32 * hb:32 * hb + 32, :]
                        nent = len(entries)
                        for i, (c, jj) in enumerate(entries):
                            s, cl = c // SEGC, c % SEGC
                            nc.tensor.matmul(
                                pms, Gs[s][:, cl, 32 * jj:32 * jj + 32],
                                OHs[s][:, cl, :],
                                start=(i == 0), stop=(i == nent - 1),
                                tile_position=(0, 32 * hb),
                            )
                        hsl = h_sb[32 * hb:32 * hb + 32, off:off + WIN]
                        nc.vector.tensor_tensor(hsl, pms, hsl, OP.add)
                # leaky relu: h = max(z, 0.01*z)
                lrt = wp.tile([128, BLK], F32, name=f"lr{l}",
                              tag="lrt", bufs=2)
                nc.vector.tensor_scalar(lrt[:], h_sb[:], NEG, None, OP.mult)
                nc.vector.tensor_tensor(h_sb[:], h_sb[:], lrt[:], OP.max)
                if l + 1 < NCONV:
                    msg_full(l + 1)
                else:
                    for b in range(NBLK):
                        pool_block(b)
                _s_sc.__exit__(None, None, None)
                if l + 1 < NCONV:
                    _s_ag1 = nc.named_scope(f"ph_ag{l+1}"); _s_ag1.__enter__()
                    nc.gpsimd.collective_compute(
                        "AllGather", OP.bypass,
                        ins=[shard[l + 1][:]], outs=[table[l + 1][:]],
                        replica_groups=AG_GROUPS,
                    )
                    _s_ag1.__exit__(None, None, None)

            # ================= pool + readout =================
            _s_po = nc.named_scope("ph_pool"); _s_po.__enter__()
            pool_sb = wp.tile([HID, NG], F32, name="pool_sb")
            nc.vector.tensor_copy(pool_sb[:], ppool[:])
            nc.sync.dma_start(pool_in[:], pool_sb[:])
            nc.gpsimd.collective_compute(
                "AllReduce", OP.add,
                ins=[pool_in[:]], outs=[pool_out[:]],
                replica_groups=AG_GROUPS,
            )
            psum_sb = wp.tile([HID, NG], F32, name="psum_sb")
            nc.sync.dma_start(psum_sb[:], pool_out[:])
            ptry = ptr_p.tile([128, 32], F32, name="ptry", tag="ptr")
            nc.tensor.transpose(ptry[:], psum_sb[:], id32_sb[0:32, :])
            y_sb = wp.tile([NG, HID], F32, name="y_sb")
            nc.vector.tensor_scalar(y_sb[:], ptry[:], invc_sb[:], None, OP.mult)

            # readout: sin(y), cos(y) via the same range reduction
            def sincos(src, pfx, quarter):
                n = wp.tile([NG, HID], I32, name=f"{pfx}n")
                nf = wp.tile([NG, HID], F32, name=f"{pfx}nf")
                if quarter:
                    nc.vector.tensor_scalar(n[:], src, INV_2PI, 0.25, OP.mult, OP.add)
                    nc.vector.tensor_scalar(nf[:], n[:], -TWO_PI, PI / 2,
                                            OP.mult, OP.add)
                else:
                    nc.vector.tensor_scalar(n[:], src, INV_2PI, None, OP.mult)
                    nc.vector.tensor_scalar(nf[:], n[:], -TWO_PI, None, OP.mult)
                r = wp.tile([NG, HID], F32, name=f"{pfx}r")
                nc.vector.tensor_tensor(r[:], src, nf[:], OP.add)
                o = wp.tile([NG, HID], F32, name=f"{pfx}o")
                nc.scalar.activation(o[:], r[:], AF.Sin, bias=zb[:])
                return o

            sin_y = sincos(y_sb[:], "sy", False)
            cos_y = sincos(y_sb[:], "cy", True)
            nc.vector.tensor_tensor(cos_y[:], cos_y[:], w0_sb[:], OP.mult)
            nc.vector.tensor_tensor(sin_y[:], sin_y[:], w1_sb[:], OP.mult)
            nc.vector.tensor_tensor(cos_y[:], cos_y[:], sin_y[:], OP.add)
            red = wp.tile([NG, 1], F32, name="red")
            nc.vector.tensor_reduce(red[:], cos_y[:], mybir.AxisListType.X, OP.add)
            o_sb = wp.tile([NG, 1], F32, name="o_sb")
            nc.scalar.activation(o_sb[:], red[:], AF.Sigmoid, bias=bout_sb[:])
            nc.sync.dma_start(out_d[:], o_sb[:])
            _s_po.__exit__(None, None, None)

    nc.compile()
    return nc


# ----------------------------------------------------------------------------
# entry point
# ----------------------------------------------------------------------------

def kernel(x, edge_index, batch, W_in, W_conv, W_out, b_out):
    global LAST_RESULTS
    x = np.asarray(x, dtype=np.float32)
    W_in = np.asarray(W_in, dtype=np.float32)
    W_conv = np.asarray(W_conv, dtype=np.float32)
    W_out = np.asarray(W_out, dtype=np.float32)
    b_out = np.asarray(b_out, dtype=np.float32)

    meta = _prep(edge_index, batch)
    perm = meta["perm"]
    x_perm = np.zeros((NTOT, INF), dtype=np.float32)
    x_perm[perm] = x
    x_pack = _pack_x(x_perm)
    win, wc, biases, w0r, w1r = _pack_weights(W_in, W_conv, W_out)

    nc = _build(meta)

    iota = np.tile(np.arange(128, dtype=np.float32)[None, :],
                   (128, 1)).astype(ml_dtypes.bfloat16)
    id32 = np.tile(np.eye(32, dtype=np.float32), (4, 1))
    id32b = np.tile(np.eye(32, dtype=ml_dtypes.bfloat16), (4, 1))
    bout_col = np.full((128, 1), float(b_out.ravel()[0]), dtype=np.float32)

    in_maps = []
    for c in range(P):
        in_maps.append({
            "x_pack": x_pack[c],
            "win_w": win,
            "wc_w": wc,
            "biases": biases,
            "w0r": w0r,
            "w1r": w1r,
            "bout": bout_col,
            "invc": meta["invc"].astype(np.float32),
            "gidx": meta["gidx_dev"][c],
            "dloc": meta["dloc_dev"][c],
            "iota": iota,
            "id32": id32,
            "id32b": id32b,
            "Bmat": meta["B_dev"][c],
        })

    import os as _os
    _tc = _os.environ.get("TRACE_CORES")
    _kw = {}
    if _tc:
        _kw = dict(trace_cores=[int(x) for x in _tc.split(",")], stitch_traces=True)
    res = run_bass_kernel_spmd(nc, in_maps, core_ids=list(range(P)), **_kw)
    LAST_RESULTS = res
    return np.asarray(res.results[0]["out"], dtype=np.float32)


# revision 28
# speedup vs baseline: 1.0451x; 1.0208x over previous
"""KA-GNN (Fourier-KAN message passing) on 8 Trainium2 NeuronCores — v3.

Sharding: nodes/edges partitioned by destination across 8 cores, with a
host-side node permutation that bin-packs nodes into 128-dst windows by
in-degree so every (core, window) has ~equal edge count. Per conv layer
each core computes its msg shard (node-wise Fourier-KAN) in bf16 packed
2-nodes-per-256B-row (node pair duplicated to satisfy the 256B gather
granularity), an AllGather builds the full msg table in DRAM, then async
dma_gather segments (rotating the 4 SWDGE queues so their drains overlap)
pull per-edge source rows. Host-precomputed one-hot matrices (DMA'd from
DRAM, no on-device IS_EQ) drive bf16 scatter-matmuls accumulating each
128-dst window in PSUM. The next layer's msg compute is emitted per
32-feature block as soon as that block's windows finish, hiding it under
the scatter. Pool via one-hot matmul + small AllReduce; readout + sigmoid
on device.

The Fourier features sin/cos(k*h), k=1..4 are built from sin(h), cos(h)
(range-reduced via round-to-nearest f32->i32 cast) plus ScalarE Square
chains; the k-harmonics are linear in 8 basis tensors, so the KAN weights
are remixed host-side onto that basis (plus a per-output bias column).
"""

import heapq
import math
import numpy as np
import ml_dtypes

import concourse.bacc as bacc
import concourse.mybir as mybir
import concourse.tile as tile
from concourse.bass_utils import run_bass_kernel_spmd

F32 = mybir.dt.float32
BF16 = mybir.dt.bfloat16
I16 = mybir.dt.int16
I32 = mybir.dt.int32
F16 = mybir.dt.float16
AF = mybir.ActivationFunctionType
OP = mybir.AluOpType

P = 8
HID = 32
INF = 64
NG = 128
NCONV = 2
NEG = 0.01

NPC = 6656                 # nodes per core (padded total 53248)
NTOT = NPC * P
NBLK = 4
BLK = NPC // NBLK          # 1664
WIN = 128
NWIN = NPC // WIN          # 52
N_NODES_REAL = 50000
NROWS = NTOT // 2          # 2 nodes per 256B bf16 row (pair duplicated)
SEGC = 16                  # chunks per gather segment (2048 tokens)

TWO_PI = float(2 * math.pi)
PI = float(math.pi)
INV_2PI = float(1.0 / (2 * math.pi))
ISQ2 = float(1.0 / math.sqrt(2.0))

LAST_RESULTS = None        # test.py reads exec_time_ns from here


def _install_ntff_hook():
    # restore the axon NTFF profiling hook when the image's antenv lacks it
    import sys
    import types
    try:
        import antenv.axon_hooks  # noqa: F401
        return
    except ImportError:
        pass
    try:
        import antenv
        from trn_agent_boot.trn_boot import _ntff_profile_via_ctypes
        hook = _ntff_profile_via_ctypes("/opt/axon/libaxon_pjrt.so")
        mod = types.ModuleType("antenv.axon_hooks")
        holder = {"h": hook}
        mod.set_axon_ntff_profile_hook = lambda h: holder.__setitem__("h", h)
        mod.get_axon_ntff_profile_hook = lambda: holder["h"]
        sys.modules["antenv.axon_hooks"] = mod
        antenv.axon_hooks = mod
    except Exception:
        pass


_install_ntff_hook()


# ----------------------------------------------------------------------------
# host-side sharding / index prep
# ----------------------------------------------------------------------------

def _balance_nodes(dst):
    """Bin-pack real nodes into P*NWIN windows of <=128 so every window's
    in-edge count is ~equal. Returns perm: old node id -> new node id."""
    indeg = np.bincount(dst, minlength=N_NODES_REAL).astype(np.int64)
    order = np.argsort(-indeg, kind="stable")
    nw = P * NWIN
    heap = [(0, 0, w) for w in range(nw)]   # (sum, count, window)
    heapq.heapify(heap)
    wslot = np.empty(N_NODES_REAL, dtype=np.int64)
    wcnt = np.zeros(nw, dtype=np.int64)
    for i in order:
        s, c, w = heapq.heappop(heap)
        wslot[i] = w
        cpos = wcnt[w]
        wcnt[w] += 1
        if c + 1 < WIN:
            heapq.heappush(heap, (s + indeg[i], c + 1, w))
        # record position later via stable counting
    # positions: stable order of assignment per window
    perm = np.empty(N_NODES_REAL, dtype=np.int64)
    pos_in_w = np.zeros(nw, dtype=np.int64)
    for i in order:
        w = wslot[i]
        core, wl = w // NWIN, w % NWIN
        perm[i] = core * NPC + wl * WIN + pos_in_w[w]
        pos_in_w[w] += 1
    return perm


def _prep(edge_index, batch):
    src0 = np.asarray(edge_index[0], dtype=np.int64)
    dst0 = np.asarray(edge_index[1], dtype=np.int64)
    bat0 = np.asarray(batch, dtype=np.int64)
    E = src0.shape[0]

    perm = _balance_nodes(dst0)
    src = perm[src0]
    dst = perm[dst0]

    core = dst // NPC
    w_in_core = (dst % NPC) // WIN
    dloc = dst % WIN
    j = src % 2                               # parity: 32-col slice of row
    row = src // 2                            # table row (node pair, dup'd)

    key = (core * NWIN + w_in_core) * 2 + j
    cnt = np.bincount(key, minlength=P * NWIN * 2).reshape(P, NWIN, 2)
    cmax = cnt.max(axis=0)                    # (NWIN, 2)
    nch = np.ceil(cmax / 128).astype(np.int64)
    nch = np.maximum(nch, (cmax > 0))

    # chunk layout per window: parity-0 chunks then parity-1 chunks
    chunk_base = np.zeros((NWIN, 2), dtype=np.int64)
    win_plan = []                             # per window: [(chunk, j), ...]
    ch = 0
    for w in range(NWIN):
        entries = []
        for jj in range(2):
            chunk_base[w, jj] = ch
            for _ in range(int(nch[w, jj])):
                entries.append((ch, jj))
                ch += 1
        win_plan.append(entries)
    CH = ch
    NSEG = (CH + SEGC - 1) // SEGC
    CHP = NSEG * SEGC
    NTOKP = CHP * 128

    # sort edges by (core, window, parity, row) -> slots
    skey = key * (NROWS + 1) + row
    order = np.argsort(skey, kind="stable")
    s_key = key[order]
    s_row = row[order]
    s_dloc = dloc[order]
    grp_start = np.zeros(P * NWIN * 2, dtype=np.int64)
    grp_start[1:] = np.cumsum(np.bincount(s_key, minlength=P * NWIN * 2))[:-1]
    pos = np.arange(E) - grp_start[s_key]

    s_core = s_key // (NWIN * 2)
    s_w = (s_key // 2) % NWIN
    s_j = s_key % 2
    slot = s_core * NTOKP + chunk_base[s_w, s_j] * 128 + pos

    tok_row = ((np.arange(P * NTOKP) * 9973) % NROWS).astype(np.int16)
    tok_row[slot] = s_row.astype(np.int16)

    ti = tok_row.reshape(P, NTOKP // 16, 16)
    ti = np.swapaxes(ti, 1, 2)                               # (P, 16, cols)
    gidx_dev = np.tile(ti, (1, 8, 1)).copy()                 # (P, 128, cols)

    # per-token dst-local ids (255 = dummy -> zero one-hot row on device)
    tok_dloc = np.full(P * NTOKP, 255.0, dtype=np.float32)
    tok_dloc[slot] = s_dloc.astype(np.float32)
    td = tok_dloc.reshape(P, CHP, 128)
    dloc_dev = np.ascontiguousarray(
        np.swapaxes(td, 1, 2)).astype(ml_dtypes.bfloat16)    # (P, 128, CHP)

    bat = np.full(N_NODES_REAL, 0, dtype=np.int64)
    bat[:] = bat0
    bat_pad = np.full(NTOT, -1, dtype=np.int64)
    bat_pad[perm] = bat
    B = (bat_pad[:, None] == np.arange(NG)[None, :])
    B_dev = np.ascontiguousarray(
        B.reshape(P, NWIN, 128, NG).transpose(0, 2, 1, 3)
        .reshape(P, 128, NWIN * NG)).astype(ml_dtypes.bfloat16)
    counts = np.bincount(bat0, minlength=NG)[:NG].astype(np.float32)
    invc = (1.0 / np.maximum(counts, 1.0)).reshape(NG, 1)

    return dict(CH=CH, CHP=CHP, NSEG=NSEG, win_plan=win_plan, perm=perm,
                gidx_dev=gidx_dev, dloc_dev=dloc_dev, B_dev=B_dev, invc=invc)


def _pack_x(x):
    xp = np.zeros((NTOT, INF), dtype=np.float32)
    xp[:x.shape[0]] = x
    xc = xp.reshape(P, 2, NPC // 2, INF)
    return np.ascontiguousarray(xc.transpose(0, 1, 3, 2).reshape(P, 128, NPC // 2))


def _remix(W):
    """W: (2, out, in, 4) -> 8 slot matrices (out, in) + bias (out,).

    Basis slots: [sin h, cos h, sin^2 h, (1+sin2h)/2, sin^2 2h,
                  (1-sin4h)/2, sin3h, cos3h]."""
    W0, W1 = W[0], W[1]          # cos / sin coefficient stacks
    slots = [
        W1[:, :, 0],
        W0[:, :, 0],
        -2.0 * W0[:, :, 1],
        2.0 * W1[:, :, 1],
        -2.0 * W0[:, :, 3],
        -2.0 * W1[:, :, 3],
        W1[:, :, 2],
        W0[:, :, 2],
    ]
    bias = (W0[:, :, 1] - W1[:, :, 1] + W0[:, :, 3] + W1[:, :, 3]).sum(axis=1)
    return slots, bias.astype(np.float32)


def _pack_weights(W_in, W_conv, W_out):
    sl_in, b_in = _remix(W_in)
    # input: fused stationary per (half, slot): 128x128 with 64x32 blocks at
    # (rows 0:64 -> out 32*half) and (rows 64:128 -> out 32*(half+2))
    win = np.zeros((128, 2 * 8 * 128), dtype=np.float16)
    for h in range(2):
        for m in range(8):
            c0 = (h * 8 + m) * 128
            win[0:64, c0 + 32 * h:c0 + 32 * h + 32] = (
                sl_in[m].T.astype(np.float16))
            win[64:128, c0 + 32 * (h + 2):c0 + 32 * (h + 2) + 32] = (
                sl_in[m].T.astype(np.float16))
    # conv: fused stationary per (layer, slot): block-diag of 4 identical
    # 32x32 tiles
    wc = np.zeros((128, NCONV * 8 * 128), dtype=np.float16)
    biases = np.zeros((128, 1 + NCONV), dtype=np.float32)
    biases[:, 0] = np.tile(b_in, 4)
    for l in range(NCONV):
        sl, bl = _remix(W_conv[l])
        biases[:, 1 + l] = np.tile(bl, 4)
        for b in range(4):
            for m in range(8):
                c0 = (l * 8 + m) * 128
                wc[32 * b:32 * b + 32, c0 + 32 * b:c0 + 32 * b + 32] = (
                    sl[m].T.astype(np.float16))
    w0r = np.tile(W_out[0, 0, :, 0].astype(np.float32), (128, 1))
    w1r = np.tile(W_out[1, 0, :, 0].astype(np.float32), (128, 1))
    return win, wc, biases, w0r, w1r


# ----------------------------------------------------------------------------
# device program
# ----------------------------------------------------------------------------

def _build(meta):
    CH, CHP, NSEG = meta["CH"], meta["CHP"], meta["NSEG"]
    win_plan = meta["win_plan"]
    XCOLS = NPC // 2                  # 3328
    NTOKP = CHP * 128

    nc = bacc.Bacc("TRN2", target_bir_lowering=False, debug=False,
                   num_devices=P, num_swdge_queues=4)

    x_d = nc.dram_tensor("x_pack", [128, XCOLS], F32, kind="ExternalInput")
    win_d = nc.dram_tensor("win_w", [128, 2 * 8 * 128], F16, kind="ExternalInput")
    wc_d = nc.dram_tensor("wc_w", [128, NCONV * 8 * 128], F16, kind="ExternalInput")
    bias_d = nc.dram_tensor("biases", [128, 1 + NCONV], F32, kind="ExternalInput")
    w0_d = nc.dram_tensor("w0r", [128, HID], F32, kind="ExternalInput")
    w1_d = nc.dram_tensor("w1r", [128, HID], F32, kind="ExternalInput")
    bout_d = nc.dram_tensor("bout", [128, 1], F32, kind="ExternalInput")
    invc_d = nc.dram_tensor("invc", [128, 1], F32, kind="ExternalInput")
    gidx_d = nc.dram_tensor("gidx", [128, NTOKP // 16], I16, kind="ExternalInput")
    dloc_d = nc.dram_tensor("dloc", [128, CHP], BF16, kind="ExternalInput")
    iota_d = nc.dram_tensor("iota", [128, 128], BF16, kind="ExternalInput")
    id32_d = nc.dram_tensor("id32", [128, 32], F32, kind="ExternalInput")
    id32b_d = nc.dram_tensor("id32b", [128, 32], BF16, kind="ExternalInput")
    B_d = nc.dram_tensor("Bmat", [128, NWIN * NG], BF16, kind="ExternalInput")

    out_d = nc.dram_tensor("out", [NG, 1], F32, kind="ExternalOutput")

    AG_GROUPS = [list(range(P))]
    NT = 416
    GBUF = 7

    with tile.TileContext(nc) as tc:
        with (
            tc.tile_pool(name="const", bufs=1) as cp,
            tc.tile_pool(name="feat", bufs=1) as fp,
            tc.tile_pool(name="ftmp", bufs=1) as tp,
            tc.tile_pool(name="work", bufs=1) as wp,
            tc.tile_pool(name="gbuf", bufs=1) as gp,
            tc.tile_pool(name="ohp", bufs=1) as ohp,
            tc.tile_pool(name="pmsg", bufs=2, space="PSUM") as pmsg_p,
            tc.tile_pool(name="ptr", bufs=2, space="PSUM") as ptr_p,
            tc.tile_pool(name="pm", bufs=3, space="PSUM") as pm_p,
            tc.tile_pool(name="ppool", bufs=1, space="PSUM") as ppool_p,
            tc.tile_pool(name="dram", bufs=1, space="DRAM") as dp,
        ):
            # ---- constants ----
            x_sb = wp.tile([128, XCOLS], F32, name="x_sb", tag="bigx")
            nc.sync.dma_start(x_sb[:], x_d[:])
            win_sb = cp.tile([128, 2 * 8 * 128], F16)
            nc.sync.dma_start(win_sb[:], win_d[:])
            wc_sb = cp.tile([128, NCONV * 8 * 128], F16)
            nc.sync.dma_start(wc_sb[:], wc_d[:])
            bias_sb = cp.tile([128, 1 + NCONV], F32)
            nc.sync.dma_start(bias_sb[:], bias_d[:])
            w0_sb = cp.tile([128, HID], F32)
            nc.sync.dma_start(w0_sb[:], w0_d[:])
            w1_sb = cp.tile([128, HID], F32)
            nc.sync.dma_start(w1_sb[:], w1_d[:])
            bout_sb = cp.tile([128, 1], F32)
            nc.sync.dma_start(bout_sb[:], bout_d[:])
            invc_sb = cp.tile([128, 1], F32)
            nc.sync.dma_start(invc_sb[:], invc_d[:])
            gidx_sb = cp.tile([128, NTOKP // 16], I16)
            nc.sync.dma_start(gidx_sb[:], gidx_d[:])
            dloc_sb = cp.tile([128, CHP], BF16)
            nc.sync.dma_start(dloc_sb[:], dloc_d[:])
            iota_sb = cp.tile([128, 128], BF16)
            nc.sync.dma_start(iota_sb[:], iota_d[:])
            id32_sb = cp.tile([128, 32], F32)
            nc.sync.dma_start(id32_sb[:], id32_d[:])
            id32b_sb = cp.tile([128, 32], BF16)
            nc.sync.dma_start(id32b_sb[:], id32b_d[:])
            zb = cp.tile([128, 1], F32)
            nc.vector.memset(zb[:], 0.0)
            m1 = cp.tile([128, 1], F32)
            nc.vector.memset(m1[:], -1.0)

            h_sb = cp.tile([128, BLK], F32)    # packed h^T: partition 32*blk+f

            shard = [dp.tile([NPC // 2, 128], BF16, name=f"shard{l}")
                     for l in range(NCONV)]
            table = [dp.tile([NROWS, 128], BF16, name=f"table{l}",
                             addr_space="Shared")
                     for l in range(NCONV)]
            pool_in = dp.tile([HID, NG], F32)
            pool_out = dp.tile([HID, NG], F32, addr_space="Shared")

            def feat_chain(src, FREE, pfx, row0, nrows, tagset=0):
                """8 f16 basis-feature tiles (rows row0:row0+nrows valid)."""
                ps = slice(row0, row0 + nrows)

                def ts(dst, a, s1, s2, o0, o1=None):
                    if o1 is None:
                        nc.vector.tensor_scalar(dst, a, s1, None, o0)
                    else:
                        nc.vector.tensor_scalar(dst, a, s1, s2, o0, o1)

                def scr(nm, dt=F32):
                    t = tp.tile([128, FREE], dt, name=f"{pfx}{nm}", tag="scr",
                                bufs=3, padded_shape=[128, BLK])
                    return t[ps, :]

                def keep(nm):
                    t = tp.tile([128, FREE], F32, name=f"{pfx}{nm}", tag=nm,
                                bufs=1, padded_shape=[128, BLK])
                    return t[ps, :]

                slots = [fp.tile([128, FREE], F16, name=f"{pfx}slot{i}",
                                 tag=f"feat{i}_{tagset}",
                                 padded_shape=[128, BLK])
                         for i in range(8)]
                sl = [s[ps, :] for s in slots]
                zbs, m1s = zb[ps, :], m1[ps, :]

                n0 = scr("n0", I32)
                ts(n0, src, INV_2PI, None, OP.mult)
                nf0 = scr("nf0")
                ts(nf0, n0, -TWO_PI, None, OP.mult)
                r0 = scr("r0")
                nc.vector.tensor_tensor(r0, src, nf0, OP.add)
                n9 = scr("n9", I32)
                ts(n9, src, INV_2PI, 0.25, OP.mult, OP.add)
                nf9 = scr("nf9")
                ts(nf9, n9, -TWO_PI, PI / 2, OP.mult, OP.add)
                r9 = scr("r9")
                nc.vector.tensor_tensor(r9, src, nf9, OP.add)

                nc.scalar.activation(sl[0], r0, AF.Sin, bias=zbs)
                nc.scalar.activation(sl[1], r9, AF.Sin, bias=zbs)
                nc.scalar.activation(sl[2], sl[0], AF.Square)
                d = scr("d")
                nc.vector.tensor_tensor(d, sl[0], sl[1], OP.add)
                nc.scalar.activation(sl[3], d, AF.Square, scale=ISQ2)
                nc.scalar.activation(sl[4], sl[3], AF.Square,
                                     bias=m1s, scale=2.0)
                tc2 = scr("tc2")
                ts(tc2, sl[2], -2.0, 1.0, OP.mult, OP.add)
                ts2 = scr("ts2")
                ts(ts2, sl[3], 2.0, -1.0, OP.mult, OP.add)
                td2 = scr("td2")
                nc.vector.tensor_tensor(td2, tc2, ts2, OP.subtract)
                nc.scalar.activation(sl[5], td2, AF.Square, scale=ISQ2)
                t3 = scr("t3", F16)
                ts(t3, sl[2], -4.0, 3.0, OP.mult, OP.add)
                nc.vector.tensor_tensor(sl[6], sl[0], t3, OP.mult)
                t4 = scr("t4", F16)
                ts(t4, sl[2], -4.0, 1.0, OP.mult, OP.add)
                nc.vector.tensor_tensor(sl[7], sl[1], t4, OP.mult)
                return slots

            def msg_full(l):
                """Emit msg compute for conv layer l (full width)."""
                mTp = wp.tile([128, BLK], BF16, name=f"mT{l}",
                              tag="mT", bufs=2)
                slots = feat_chain(h_sb[:], BLK, f"m{l}_", 0, 128)
                msg_matmuls(l, slots, mTp, 0)
                msg_stage(l, mTp)

            def msg_matmuls(l, slots, mTp, c0):
                for t4i in range(4):
                    pm2 = pmsg_p.tile([128, NT], F32, name="pm2", tag="pmsg")
                    for m in range(8):
                        nc.tensor.matmul(
                            pm2[:],
                            wc_sb[:, (l * 8 + m) * 128:(l * 8 + m + 1) * 128],
                            slots[m][:, NT * t4i:NT * (t4i + 1)],
                            start=(m == 0), stop=(m == 7),
                        )
                    nc.vector.tensor_scalar(
                        mTp[:, c0 + NT * t4i:c0 + NT * (t4i + 1)], pm2[:],
                        bias_sb[:, 1 + l:2 + l], None, OP.add)

            def msg_stage(l, mTp):
                # reorder columns: even nodes first, then odd, per w2 tile
                mTe = wp.tile([128, BLK], BF16, name=f"mTe{l}",
                              tag="mTe", bufs=2)
                nc.vector.tensor_copy(
                    mTe[:].rearrange("p (w2 sub rh) -> p w2 sub rh",
                                     w2=13, sub=2),
                    mTp[:].rearrange("p (w2 rh sub) -> p w2 sub rh",
                                     w2=13, sub=2))
                for b in range(NBLK):
                    ps = slice(32 * b, 32 * b + 32)
                    stage = wp.tile([128, 13 * HID], BF16, name=f"stg{l}_{b}",
                                    tag="stage", bufs=2)
                    for w2 in range(13):
                        ptr = ptr_p.tile([128, 32], BF16, name="ptrt", tag="ptr")
                        nc.tensor.transpose(
                            ptr[:], mTe[ps, 128 * w2:128 * (w2 + 1)],
                            id32b_sb[ps, :],
                            tile_position=(32 * b, 0))
                        nc.scalar.activation(stage[:, 32 * w2:32 * (w2 + 1)],
                                             ptr[:], AF.Copy)
                    # partitions 0:64 = even rows, 64:128 = odd rows
                    for sub in range(2):
                        nc.sync.dma_start(
                            shard[l][832 * b:832 * (b + 1),
                                     32 * sub:32 * sub + 32].rearrange(
                                "(w2 rh) f -> rh w2 f", w2=13),
                            stage[64 * sub:64 * sub + 64, :].rearrange(
                                "p (w2 f) -> p w2 f", f=HID),
                        )

            # ================= input KAN: x -> h =================
            _s_in = nc.named_scope("ph_input"); _s_in.__enter__()
            for half in range(2):
                xsl = x_sb[:, half * BLK:(half + 1) * BLK]
                slots = feat_chain(xsl, BLK, f"x{half}_", 0, 128,
                                   tagset=half)
                for t4i in range(BLK // NT):
                    off = NT * t4i
                    ph = pmsg_p.tile([128, NT], F32, name="ph", tag="pmsg")
                    for m in range(8):
                        nc.tensor.matmul(
                            ph[:],
                            win_sb[:, (half * 8 + m) * 128:(half * 8 + m + 1) * 128],
                            slots[m][:, NT * t4i:NT * (t4i + 1)],
                            start=(m == 0), stop=(m == 7),
                        )
                    for hb in (half, half + 2):
                        nc.vector.tensor_scalar(
                            h_sb[32 * hb:32 * hb + 32, off:off + NT],
                            ph[32 * hb:32 * hb + 32, :],
                            bias_sb[32 * hb:32 * hb + 32, 0:1], None, OP.add)
            msg_full(0)
            _s_in.__exit__(None, None, None)

            # pool resources (consumed per-block during the last scatter)
            B_sb = wp.tile([128, NWIN * NG], BF16, name="B_sb", tag="bigx")
            nc.sync.dma_start(B_sb[:], B_d[:])
            ppool = ppool_p.tile([HID, NG], F32)
            hbf = wp.tile([128, BLK], BF16, name="hbf", tag="hbf", bufs=1)

            def pool_block(b):
                ps = slice(32 * b, 32 * b + 32)
                nc.vector.tensor_copy(hbf[ps, :], h_sb[ps, :])
                for w2 in range(13):
                    w = 13 * b + w2
                    ptb = ptr_p.tile([128, 32], BF16, name="ptb", tag="ptr")
                    nc.tensor.transpose(
                        ptb[:], hbf[ps, 128 * w2:128 * (w2 + 1)],
                        id32b_sb[ps, :], tile_position=(32 * b, 0))
                    htile = wp.tile([128, 32], BF16, name="htile",
                                    tag="htile", bufs=3)
                    nc.scalar.activation(htile[:], ptb[:], AF.Copy)
                    nc.tensor.matmul(
                        ppool[:], htile[:], B_sb[:, NG * w:NG * (w + 1)],
                        start=(w == 0), stop=(w == NWIN - 1),
                    )

            _s_ag = nc.named_scope("ph_ag0"); _s_ag.__enter__()
            nc.gpsimd.collective_compute(
                "AllGather", OP.bypass,
                ins=[shard[0][:]], outs=[table[0][:]],
                replica_groups=AG_GROUPS,
            )
            _s_ag.__exit__(None, None, None)

            # ================= conv layers: scatter (+ pipelined msg) ======
            for l in range(NCONV):
                _s_sc = nc.named_scope(f"ph_scat{l}"); _s_sc.__enter__()
                Gs = [None] * NSEG
                OHs = [None] * NSEG
                state = {"issued": 0, "built": 0}

                def issue_seg(s, l=l, Gs=Gs):
                    G = gp.tile([128, SEGC, 128], BF16, name=f"G{l}_{s}",
                                tag=f"G{s % GBUF}")
                    nc.gpsimd.dma_gather(
                        G[:], table[l][:],
                        gidx_sb[:, s * (SEGC * 8):(s + 1) * (SEGC * 8)],
                        num_idxs=SEGC * 128, num_idxs_reg=SEGC * 128,
                        elem_size=128, single_packet=False, queue_num=s % 4,
                    )
                    Gs[s] = G

                iota_b = iota_sb[:].rearrange("p (x d) -> p x d", x=1)

                def load_oh(s, l=l, OHs=OHs):
                    oh = ohp.tile([128, SEGC, 128], BF16, name=f"oh{l}_{s}",
                                  tag=f"oh{s % 4}")
                    for hseg in range(2):
                        c0 = s * SEGC + hseg * (SEGC // 2)
                        nc.vector.tensor_tensor(
                            oh[:, hseg * (SEGC // 2):(hseg + 1) * (SEGC // 2), :],
                            iota_b.to_broadcast([128, SEGC // 2, 128]),
                            dloc_sb[:, c0:c0 + SEGC // 2]
                            .to_broadcast([128, SEGC // 2, 128]),
                            OP.is_equal)
                    OHs[s] = oh

                for w in range(NWIN):
                    entries = win_plan[w]
                    if entries:
                        cg = entries[0][0]
                        g_need = min(cg // SEGC + (GBUF - 1), NSEG - 1)
                        while state["issued"] <= g_need:
                            issue_seg(state["issued"])
                            state["issued"] += 1
                        oh_need = min((cg + len(entries) - 1) // SEGC + 1,
                                      NSEG - 1)
                        while state["built"] <= oh_need:
                            load_oh(state["built"])
                            state["built"] += 1
                        hb, off = (w * WIN) // BLK, (w * WIN) % BLK
                        pm = pm_p.tile([128, WIN], F32, name="pmw", tag="pm")
                        pms = pm[32 * hb:32 * hb + 32, :]
                        nent = len(entries)
                        for i, (c, jj) in enumerate(entries):
                            s, cl = c // SEGC, c % SEGC
                            nc.tensor.matmul(
                                pms, Gs[s][:, cl, 32 * jj:32 * jj + 32],
                                OHs[s][:, cl, :],
                                start=(i == 0), stop=(i == nent - 1),
                                tile_position=(0, 32 * hb),
                            )
                        hsl = h_sb[32 * hb:32 * hb + 32, off:off + WIN]
                        nc.vector.tensor_tensor(hsl, pms, hsl, OP.add)
                # leaky relu: h = max(z, 0.01*z)
                lrt = wp.tile([128, BLK], F32, name=f"lr{l}",
                              tag="lrt", bufs=2)
                nc.vector.tensor_scalar(lrt[:], h_sb[:], NEG, None, OP.mult)
                nc.vector.tensor_tensor(h_sb[:], h_sb[:], lrt[:], OP.max)
                if l + 1 < NCONV:
                    msg_full(l + 1)
                else:
                    for b in range(NBLK):
                        pool_block(b)
                _s_sc.__exit__(None, None, None)
                if l + 1 < NCONV:
                    _s_ag1 = nc.named_scope(f"ph_ag{l+1}"); _s_ag1.__enter__()
                    nc.gpsimd.collective_compute(
                        "AllGather", OP.bypass,
                        ins=[shard[l + 1][:]], outs=[table[l + 1][:]],
                        replica_groups=AG_GROUPS,
                    )
                    _s_ag1.__exit__(None, None, None)

            # ================= pool + readout =================
            _s_po = nc.named_scope("ph_pool"); _s_po.__enter__()
            pool_sb = wp.tile([HID, NG], F32, name="pool_sb")
            nc.vector.tensor_copy(pool_sb[:], ppool[:])
            nc.sync.dma_start(pool_in[:], pool_sb[:])
            nc.gpsimd.collective_compute(
                "AllReduce", OP.add,
                ins=[pool_in[:]], outs=[pool_out[:]],
                replica_groups=AG_GROUPS,
            )
            psum_sb = wp.tile([HID, NG], F32, name="psum_sb")
            nc.sync.dma_start(psum_sb[:], pool_out[:])
            ptry = ptr_p.tile([128, 32], F32, name="ptry", tag="ptr")
            nc.tensor.transpose(ptry[:], psum_sb[:], id32_sb[0:32, :])
            y_sb = wp.tile([NG, HID], F32, name="y_sb")
            nc.vector.tensor_scalar(y_sb[:], ptry[:], invc_sb[:], None, OP.mult)

            # readout: sin(y), cos(y) via the same range reduction
            def sincos(src, pfx, quarter):
                n = wp.tile([NG, HID], I32, name=f"{pfx}n")
                nf = wp.tile([NG, HID], F32, name=f"{pfx}nf")
                if quarter:
                    nc.vector.tensor_scalar(n[:], src, INV_2PI, 0.25, OP.mult, OP.add)
                    nc.vector.tensor_scalar(nf[:], n[:], -TWO_PI, PI / 2,
                                            OP.mult, OP.add)
                else:
                    nc.vector.tensor_scalar(n[:], src, INV_2PI, None, OP.mult)
                    nc.vector.tensor_scalar(nf[:], n[:], -TWO_PI, None, OP.mult)
                r = wp.tile([NG, HID], F32, name=f"{pfx}r")
                nc.vector.tensor_tensor(r[:], src, nf[:], OP.add)
                o = wp.tile([NG, HID], F32, name=f"{pfx}o")
                nc.scalar.activation(o[:], r[:], AF.Sin, bias=zb[:])
                return o

            sin_y = sincos(y_sb[:], "sy", False)
            cos_y = sincos(y_sb[:], "cy", True)
            nc.vector.tensor_tensor(cos_y[:], cos_y[:], w0_sb[:], OP.mult)
            nc.vector.tensor_tensor(sin_y[:], sin_y[:], w1_sb[:], OP.mult)
            nc.vector.tensor_tensor(cos_y[:], cos_y[:], sin_y[:], OP.add)
            red = wp.tile([NG, 1], F32, name="red")
            nc.vector.tensor_reduce(red[:], cos_y[:], mybir.AxisListType.X, OP.add)
            o_sb = wp.tile([NG, 1], F32, name="o_sb")
            nc.scalar.activation(o_sb[:], red[:], AF.Sigmoid, bias=bout_sb[:])
            nc.sync.dma_start(out_d[:], o_sb[:])
            _s_po.__exit__(None, None, None)

    nc.compile()
    return nc


# ----------------------------------------------------------------------------
# entry point
# ----------------------------------------------------------------------------

def kernel(x, edge_index, batch, W_in, W_conv, W_out, b_out):
    global LAST_RESULTS
    x = np.asarray(x, dtype=np.float32)
    W_in = np.asarray(W_in, dtype=np.float32)
    W_conv = np.asarray(W_conv, dtype=np.float32)
    W_out = np.asarray(W_out, dtype=np.float32)
    b_out = np.asarray(b_out, dtype=np.float32)

    meta = _prep(edge_index, batch)
    perm = meta["perm"]
    x_perm = np.zeros((NTOT, INF), dtype=np.float32)
    x_perm[perm] = x
    x_pack = _pack_x(x_perm)
    win, wc, biases, w0r, w1r = _pack_weights(W_in, W_conv, W_out)

    nc = _build(meta)

    iota = np.tile(np.arange(128, dtype=np.float32)[None, :],
                   (128, 1)).astype(ml_dtypes.bfloat16)
    id32 = np.tile(np.eye(32, dtype=np.float32), (4, 1))
    id32b = np.tile(np.eye(32, dtype=ml_dtypes.bfloat16), (4, 1))
    bout_col = np.full((128, 1), float(b_out.ravel()[0]), dtype=np.float32)

    in_maps = []
    for c in range(P):
        in_maps.append({
            "x_pack": x_pack[c],
            "win_w": win,
            "wc_w": wc,
            "biases": biases,
            "w0r": w0r,
            "w1r": w1r,
            "bout": bout_col,
            "invc": meta["invc"].astype(np.float32),
            "gidx": meta["gidx_dev"][c],
            "dloc": meta["dloc_dev"][c],
            "iota": iota,
            "id32": id32,
            "id32b": id32b,
            "Bmat": meta["B_dev"][c],
        })

    import os as _os
    _tc = _os.environ.get("TRACE_CORES")
    _kw = {}
    if _tc:
        _kw = dict(trace_cores=[int(x) for x in _tc.split(",")], stitch_traces=True)
    res = run_bass_kernel_spmd(nc, in_maps, core_ids=list(range(P)), **_kw)
    LAST_RESULTS = res
    return np.asarray(res.results[0]["out"], dtype=np.float32)
